# revision 7
# baseline (speedup 1.0000x reference)
"""Trainium2 Bass kernel for nn_DM_35210141892754 (4-direction VMamba block).

Sharding: 8 cores = B(2) x directions(4); each core processes one gathered
sequence (C=64, L=8192) end-to-end. Gather/scatter (reference _rcds/_merge)
run on host with numpy strided ops; all FLOPs run on device.
"""

import sys
import math

sys.path.insert(0, "/opt/trn_rl_repo")

import numpy as np

B, C, HIMG, WIMG = 2, 64, 128, 128
DEPTH = 4
D_STATE, D_CONV, EXPAND = 16, 4, 2
D_INNER = EXPAND * C  # 128
DT_RANK = math.ceil(C / 16)  # 4
L = 8192  # sequence length for step_size=2
T = 512  # device chunk size
NCH = L // T
EPS = 1e-5

OFF = {0: ((1, 0), (0, 0), (0, 1), (1, 1)),
       1: ((0, 0), (1, 0), (1, 1), (0, 1)),
       2: ((0, 1), (1, 1), (1, 0), (0, 0)),
       3: ((1, 1), (0, 1), (0, 0), (1, 0))}

_PROG = {}


# ----------------------------------------------------------------------------
# host-side gather (reference _rcds) and scatter (reference _merge), numpy
# ----------------------------------------------------------------------------

def _rcds_np(f0, f1, s, i):
    fr = np.concatenate([f0, f1], axis=3)
    fl = np.concatenate([f1, f0], axis=3)
    fb = np.concatenate([f0, f1], axis=2)
    ft = np.concatenate([f1, f0], axis=2)
    Bb, Cc = fr.shape[:2]
    r, l, b, t = OFF[i]
    y0 = fr[:, :, r[0]::s, r[1]::s].transpose(0, 1, 3, 2).reshape(Bb, Cc, -1)
    y1 = fl[:, :, l[0]::s, l[1]::s].transpose(0, 1, 3, 2).reshape(Bb, Cc, -1)[:, :, ::-1]
    y2 = fb[:, :, b[0]::s, b[1]::s].reshape(Bb, Cc, -1)
    y3 = ft[:, :, t[0]::s, t[1]::s].reshape(Bb, Cc, -1)[:, :, ::-1]
    feats = np.stack([y0, y1, y2, y3], axis=1)  # (B, 4, C, L)
    return np.ascontiguousarray(feats), fr.shape[2], fb.shape[3]


def _merge_np(ys, ori_h, ori_w, s, i):
    # ys: (B, 4, C, L)
    Bb, K, Cc, Ll = ys.shape
    Hh = -(-ori_h // s)
    Ww = -(-ori_w // s)
    nh, nw = Hh * s, Ww * s
    r, l, b, t = OFF[i]
    y2wr = np.zeros((Bb, Cc, nh, 2 * nw), ys.dtype)
    y2wl = np.zeros((Bb, Cc, nh, 2 * nw), ys.dtype)
    y2hb = np.zeros((Bb, Cc, 2 * nh, nw), ys.dtype)
    y2ht = np.zeros((Bb, Cc, 2 * nh, nw), ys.dtype)
    y2wr[:, :, r[0]::s, r[1]::s] = ys[:, 0].reshape(Bb, Cc, 2 * Ww, Hh).transpose(0, 1, 3, 2)
    y2wl[:, :, l[0]::s, l[1]::s] = ys[:, 1][:, :, ::-1].reshape(Bb, Cc, 2 * Ww, Hh).transpose(0, 1, 3, 2)
    y2hb[:, :, b[0]::s, b[1]::s] = ys[:, 2].reshape(Bb, Cc, 2 * Hh, Ww)
    y2ht[:, :, t[0]::s, t[1]::s] = ys[:, 3][:, :, ::-1].reshape(Bb, Cc, 2 * Hh, Ww)
    if ori_h != nh or ori_w != nw:
        y2wr = y2wr[:, :, :ori_h, :ori_w]
        y2wl = y2wl[:, :, :ori_h, :ori_w]
        y2ht = y2ht[:, :, :ori_h, :ori_w]
        y2hb = y2hb[:, :, :ori_h, :ori_w]
    d0r, d1r = np.split(y2wr, 2, axis=3)
    d1l, d0l = np.split(y2wl, 2, axis=3)
    d0b, d1b = np.split(y2hb, 2, axis=2)
    d1t, d0t = np.split(y2ht, 2, axis=2)
    return d0r + d0l + d0b + d0t, d1r + d1l + d1b + d1t


# ----------------------------------------------------------------------------
# device program
# ----------------------------------------------------------------------------

def _build_program():
    import concourse.bacc as bacc
    import concourse.mybir as mybir
    import concourse.tile as tile

    dt_ = mybir.dt
    F32, BF16, F32R = dt_.float32, dt_.bfloat16, dt_.float32r
    AF = mybir.ActivationFunctionType
    OP = mybir.AluOpType

    def r32(ap):
        return ap.bitcast(F32R)

    nc = bacc.Bacc("TRN2", target_bir_lowering=False, debug=False)

    def din(name, shape, d=F32):
        return nc.dram_tensor(name, shape, d, kind="ExternalInput")

    x_d = din("x", [C, L])
    W2T_d = din("W2T", [C, 2 * D_INNER])
    w1n_d = din("w1n", [1, 2 * D_INNER])
    diag_d = din("diag", [D_INNER, 4 * D_INNER])
    cbn_d = din("cbn", [D_INNER, 1])
    cbp_d = din("cbp", [D_INNER, 1])
    bzn_d = din("bzn", [D_INNER, 1])
    bzp_d = din("bzp", [D_INNER, 1])
    WxpA_d = din("WxpA", [D_INNER, DT_RANK])
    WxpB_d = din("WxpB", [D_INNER, 2 * D_STATE])
    WdtT_d = din("WdtT", [DT_RANK, D_INNER])
    dtb_d = din("dtb", [D_INNER, 1])
    Aneg_d = din("Aneg", [D_INNER, D_STATE])
    Dsk_d = din("Dsk", [D_INNER, 1])
    WoutT_d = din("WoutT", [D_INNER, C])
    Wfc1T_d = din("Wfc1T", [C, 4 * C])
    w1f_d = din("w1f", [1, 4 * C])
    bfc1_d = din("bfc1", [4 * C // 2, 2])
    Wfc2a_d = din("Wfc2a", [2 * C, C])
    Wfc2b_d = din("Wfc2b", [2 * C, C])
    mv_d = din("mv", [C, 1])
    ones1_d = din("ones1", [1, C])
    epsb_d = din("epsb", [16, 1])
    I128_d = din("I128", [D_INNER, D_INNER], dt_.bfloat16)

    out_d = nc.dram_tensor("out", [C, L], F32, kind="ExternalOutput")

    NS = D_STATE

    with tile.TileContext(nc) as tc:
        with tc.tile_pool(name="pers", bufs=1) as pers, \
             tc.tile_pool(name="wka", bufs=2) as wka, \
             tc.tile_pool(name="wkb", bufs=3) as wkb, \
             tc.tile_pool(name="wk2", bufs=1) as wk2, \
             tc.tile_pool(name="hp", bufs=2) as hp, \
             tc.tile_pool(name="cvp", bufs=2) as cvp, \
             tc.tile_pool(name="bgp", bufs=3) as bgp, \
             tc.tile_pool(name="dap", bufs=2) as dap, \
             tc.tile_pool(name="dbp", bufs=3) as dbp, \
             tc.tile_pool(name="ps", bufs=5, space="PSUM") as ps, \
             tc.tile_pool(name="pacc", bufs=2, space="PSUM") as pacc, \
             tc.tile_pool(name="dr", bufs=1, space="DRAM") as dr:

            def ld(dram, shape, d=F32, tag=None, rr=False):
                t_ = pers.tile(shape, F32R if rr else d, tag=tag)
                if rr:
                    nc.sync.dma_start(t_[:], r32(dram[:]))
                else:
                    nc.sync.dma_start(t_[:], dram[:])
                return t_

            W2T = ld(W2T_d, [C, 2 * D_INNER], tag="W2T", rr=True)
            w1n = ld(w1n_d, [1, 2 * D_INNER], tag="w1n", rr=True)
            diag = ld(diag_d, [D_INNER, 4 * D_INNER], tag="diag", rr=True)
            cbn = ld(cbn_d, [D_INNER, 1], tag="cbn")
            cbp = ld(cbp_d, [D_INNER, 1], tag="cbp")
            bzn = ld(bzn_d, [D_INNER, 1], tag="bzn")
            bzp = ld(bzp_d, [D_INNER, 1], tag="bzp")
            WxpA = ld(WxpA_d, [D_INNER, DT_RANK], tag="WxpA", rr=True)
            WxpB = ld(WxpB_d, [D_INNER, 2 * D_STATE], tag="WxpB", rr=True)
            WdtT = ld(WdtT_d, [DT_RANK, D_INNER], tag="WdtT", rr=True)
            dtb = ld(dtb_d, [D_INNER, 1], tag="dtb")
            Aneg = ld(Aneg_d, [D_INNER, D_STATE], tag="Aneg")
            Dsk = ld(Dsk_d, [D_INNER, 1], tag="Dsk")
            WoutT = ld(WoutT_d, [D_INNER, C], tag="WoutT", rr=True)
            Wfc1T = ld(Wfc1T_d, [C, 4 * C], tag="Wfc1T", rr=True)
            w1f = ld(w1f_d, [1, 4 * C], tag="w1f", rr=True)
            bfc1 = ld(bfc1_d, [4 * C // 2, 2], tag="bfc1")
            Wfc2a = ld(Wfc2a_d, [2 * C, C], tag="Wfc2a", rr=True)
            Wfc2b = ld(Wfc2b_d, [2 * C, C], tag="Wfc2b", rr=True)
            mv = ld(mv_d, [C, 1], tag="mv", rr=True)
            ones1 = ld(ones1_d, [1, C], tag="ones1", rr=True)
            epsb = ld(epsb_d, [16, 1], tag="epsb")
            I128 = ld(I128_d, [D_INNER, D_INNER], BF16, tag="I128")

            fbuf = pers.tile([C, L], F32, tag="fbuf")

            st1_dram = dr.tile([32, T], F32, tag="st1d")
            r1_dram = dr.tile([1, L], F32, tag="r1d")
            q1_dram = dr.tile([1, L], F32, tag="q1d")
            st2_dram = dr.tile([32, T], F32, tag="st2d")
            r2_dram = dr.tile([1, L], F32, tag="r2d")
            q2_dram = dr.tile([1, L], F32, tag="q2d")
            bc_dram = dr.tile([2 * NS, L], BF16, tag="bcd")

            # ================= phase 0: LN1 stats =================
            for j in range(NCH):
                sl = slice(j * T, (j + 1) * T)
                xj = wka.tile([C, T], F32, tag="xj")
                nc.sync.dma_start(r32(xj[:]), r32(x_d[:, sl]))
                x2 = wka.tile([C, T], F32, tag="f2")
                nc.scalar.activation(r32(x2[:]), xj[:], AF.Square)
                mu_ps = ps.tile([1, T], F32, tag="ps")
                nc.tensor.matmul(mu_ps[:], mv[:], r32(xj[:]), start=True, stop=True)
                ms_ps = ps.tile([1, T], F32, tag="ps")
                nc.tensor.matmul(ms_ps[:], mv[:], r32(x2[:]), start=True, stop=True)
                mu_sb = wka.tile([1, T], F32, tag="msb")
                nc.scalar.copy(mu_sb[:], mu_ps[:])
                ms_sb = wka.tile([1, T], F32, tag="ssb")
                nc.scalar.copy(ms_sb[:], ms_ps[:])
                nc.sync.dma_start(st1_dram[j:j + 1, :], mu_sb[:])
                nc.sync.dma_start(st1_dram[16 + j:17 + j, :], ms_sb[:])

            def stats_finish(st_dram, rf_dram, qf_dram):
                mu_all = wk2.tile([NCH, T], F32, tag="sa")
                nc.sync.dma_start(mu_all[:], st_dram[0:16, :])
                ms_all = wk2.tile([NCH, T], F32, tag="sb")
                nc.sync.dma_start(ms_all[:], st_dram[16:32, :])
                t1 = wk2.tile([NCH, T], F32, tag="sc")
                nc.scalar.activation(t1[:], mu_all[:], AF.Square)
                t2 = wk2.tile([NCH, T], F32, tag="sd")
                nc.vector.tensor_sub(t2[:], ms_all[:], t1[:])
                t3 = wk2.tile([NCH, T], F32, tag="sc")
                nc.scalar.activation(t3[:], t2[:], AF.Ln, bias=epsb[:])
                rstd = wk2.tile([NCH, T], F32, tag="sd")
                nc.scalar.activation(rstd[:], t3[:], AF.Exp, bias=0.0, scale=-0.5)
                mq = wk2.tile([NCH, T], F32, tag="sc")
                nc.vector.tensor_mul(mq[:], mu_all[:], rstd[:])
                nc.sync.dma_start(rf_dram[:].rearrange("a (c t) -> (a c) t", t=T), rstd[:])
                nc.sync.dma_start(qf_dram[:].rearrange("a (c t) -> (a c) t", t=T), mq[:])

            stats_finish(st1_dram, r1_dram, q1_dram)

            # ================= steady phase =================
            G = 2  # states per broadcast/mult group
            NG = NS // G
            prev_h = None
            prev_cv = None
            for j in range(NCH):
                sl = slice(j * T, (j + 1) * T)
                xj = wka.tile([C, T], F32, tag="xj")
                nc.sync.dma_start(xj[:], x_d[:, sl])
                rft = wka.tile([1, T], F32, tag="rft")
                nc.sync.dma_start(r32(rft[:]), r32(r1_dram[0:1, sl]))
                qft = wka.tile([1, T], F32, tag="qft")
                nc.sync.dma_start(r32(qft[:]), r32(q1_dram[0:1, sl]))

                rb = ps.tile([C, T], F32, tag="ps")
                nc.tensor.matmul(rb[:], ones1[:], r32(rft[:]), start=True, stop=True)
                xr = wka.tile([C, T], F32, tag="xr")
                nc.vector.tensor_mul(r32(xr[:]), xj[:], rb[:])

                xzh = ps.tile([D_INNER, T], F32, tag="ps")
                nc.tensor.matmul(xzh[:], r32(W2T[:, 0:D_INNER]), r32(xr[:]),
                                 start=True, stop=False, skip_group_check=True)
                nc.tensor.matmul(xzh[:], r32(w1n[:, 0:D_INNER]), r32(qft[:]),
                                 start=False, stop=True, skip_group_check=True)
                xzz = ps.tile([D_INNER, T], F32, tag="ps")
                nc.tensor.matmul(xzz[:], r32(W2T[:, D_INNER:2 * D_INNER]), r32(xr[:]),
                                 start=True, stop=False, skip_group_check=True)
                nc.tensor.matmul(xzz[:], r32(w1n[:, D_INNER:2 * D_INNER]), r32(qft[:]),
                                 start=False, stop=True, skip_group_check=True)

                # silu(z) with sigmoid = exp(-ln(1+e^-u)): stays in Exp/Ln table
                ez = wka.tile([D_INNER, T], F32, tag="tmpa")
                nc.scalar.activation(ez[:], xzz[:], AF.Exp, bias=bzn[:], scale=-1.0)
                nc.gpsimd.tensor_scalar_add(ez[:], ez[:], 1.0)
                lz = wka.tile([D_INNER, T], F32, tag="tmpb")
                nc.scalar.activation(lz[:], ez[:], AF.Ln)
                gz = wka.tile([D_INNER, T], F32, tag="tmpa")
                nc.scalar.activation(gz[:], lz[:], AF.Exp, bias=0.0, scale=-1.0)
                sz = wka.tile([D_INNER, T], F32, tag="sz")
                nc.vector.scalar_tensor_tensor(sz[:], xzz[:], bzp[:], gz[:], OP.add, OP.mult)

                cv = cvp.tile([D_INNER, T + 3], F32, tag="cv")
                if j == 0:
                    nc.vector.memset(cv[:, 0:3], 0.0)
                else:
                    nc.vector.tensor_copy(r32(cv[:, 0:3]), r32(prev_cv[:, T:T + 3]))
                nc.scalar.activation(r32(cv[:, 3:T + 3]), xzh[:], AF.Copy)
                cps = ps.tile([D_INNER, T], F32, tag="ps")
                for k in range(4):
                    nc.tensor.matmul(cps[:], r32(diag[:, k * D_INNER:(k + 1) * D_INNER]),
                                     r32(cv[:, k:k + T]),
                                     start=(k == 0), stop=(k == 3), skip_group_check=True)
                prev_cv = cv
                # silu(conv): same sigmoid identity
                ec = wka.tile([D_INNER, T], F32, tag="tmpc")
                nc.scalar.activation(ec[:], cps[:], AF.Exp, bias=cbn[:], scale=-1.0)
                nc.gpsimd.tensor_scalar_add(ec[:], ec[:], 1.0)
                lc = wka.tile([D_INNER, T], F32, tag="tmpd")
                nc.scalar.activation(lc[:], ec[:], AF.Ln)
                gc = wka.tile([D_INNER, T], F32, tag="tmpc")
                nc.scalar.activation(gc[:], lc[:], AF.Exp, bias=0.0, scale=-1.0)
                xh = wka.tile([D_INNER, T], F32, tag="xh")
                nc.vector.scalar_tensor_tensor(r32(xh[:]), cps[:], cbp[:], gc[:],
                                               OP.add, OP.mult)

                dblA = ps.tile([DT_RANK, T], F32, tag="ps")
                nc.tensor.matmul(dblA[:], r32(WxpA[:]), r32(xh[:]), start=True, stop=True)
                dblB = ps.tile([2 * NS, T], F32, tag="ps")
                nc.tensor.matmul(dblB[:], r32(WxpB[:]), r32(xh[:]), start=True, stop=True)
                dtin = wkb.tile([DT_RANK, T], F32, tag="dtin")
                nc.scalar.activation(r32(dtin[:]), dblA[:], AF.Copy)
                bc = wkb.tile([2 * NS, T], BF16, tag="bc")
                nc.scalar.copy(bc[:], dblB[:])
                nc.sync.dma_start(bc_dram[:, sl], bc[:])

                # broadcast B/C state-rows to all partitions, G states per DMA
                Bgs, Cgs = [], []
                for g in range(NG):
                    Bg = bgp.tile([D_INNER, G, T], BF16, tag="Bg")
                    nc.sync.dma_start(
                        Bg[:], bc_dram[g * G:(g + 1) * G, sl][None]
                        .broadcast_to([D_INNER, G, T]))
                    Bgs.append(Bg)
                    Cg = bgp.tile([D_INNER, G, T], BF16, tag="Cg")
                    nc.sync.dma_start(
                        Cg[:], bc_dram[NS + g * G:NS + (g + 1) * G, sl][None]
                        .broadcast_to([D_INNER, G, T]))
                    Cgs.append(Cg)

                dtp = ps.tile([D_INNER, T], F32, tag="ps")
                nc.tensor.matmul(dtp[:], r32(WdtT[:]), r32(dtin[:]), start=True, stop=True)
                esp = wka.tile([D_INNER, T], F32, tag="tmpa")
                nc.scalar.activation(esp[:], dtp[:], AF.Exp, bias=dtb[:], scale=1.0)
                nc.vector.tensor_scalar_add(esp[:], esp[:], 1.0)
                dt = wka.tile([D_INNER, T], F32, tag="dt")
                nc.scalar.activation(dt[:], esp[:], AF.Ln)

                w = wkb.tile([D_INNER, T], BF16, tag="w")
                nc.vector.tensor_mul(w[:], dt[:], xh[:])

                cur_h = hp.tile([D_INNER, NS, T], BF16, tag="h")
                yacc = pacc.tile([D_INNER, T], F32, tag="yacc")
                for g in range(NG):
                    dAg = dap.tile([D_INNER, G, T], F32, tag="dAg")
                    for k in range(G):
                        n = g * G + k
                        nc.scalar.activation(dAg[:, k, :], dt[:], AF.Exp, bias=0.0,
                                             scale=Aneg[:, n:n + 1])
                    dBg = dbp.tile([D_INNER, G, T], BF16, tag="dBg")
                    nc.vector.tensor_tensor(dBg[:], w[:][:, None].broadcast_to(
                        [D_INNER, G, T]), Bgs[g][:], OP.mult)
                    for k in range(G):
                        n = g * G + k
                        init = 0.0 if j == 0 else prev_h[:, n, T - 1:T]
                        nc.vector.tensor_tensor_scan(cur_h[:, n, :], dAg[:, k, :],
                                                     dBg[:, k, :], init,
                                                     OP.mult, OP.add)
                    Pg = dbp.tile([D_INNER, G, T], BF16, tag="Pg")
                    if g % 2 == 0:
                        nc.gpsimd.tensor_tensor(Pg[:], cur_h[:, g * G:(g + 1) * G, :],
                                                Cgs[g][:], OP.mult)
                    else:
                        nc.vector.tensor_tensor(Pg[:], cur_h[:, g * G:(g + 1) * G, :],
                                                Cgs[g][:], OP.mult)
                    for k in range(G):
                        n = g * G + k
                        nc.tensor.matmul(yacc[:], I128[:], Pg[:, k, :], start=(n == 0),
                                         stop=(n == NS - 1), skip_group_check=True)
                prev_h = cur_h

                y = wka.tile([D_INNER, T], F32, tag="y")
                nc.vector.scalar_tensor_tensor(y[:], xh[:], Dsk[:], yacc[:],
                                               OP.mult, OP.add)
                gated = wka.tile([D_INNER, T], F32, tag="gated")
                nc.vector.tensor_mul(r32(gated[:]), y[:], sz[:])
                opj = ps.tile([C, T], F32, tag="ps")
                nc.tensor.matmul(opj[:], r32(WoutT[:]), r32(gated[:]), start=True, stop=True)
                nc.vector.tensor_add(r32(fbuf[:, sl]), xj[:], opj[:])

                f2 = wka.tile([C, T], F32, tag="f2")
                nc.scalar.activation(r32(f2[:]), fbuf[:, sl], AF.Square)
                mu2_ps = ps.tile([1, T], F32, tag="ps")
                nc.tensor.matmul(mu2_ps[:], mv[:], r32(fbuf[:, sl]), start=True, stop=True)
                ms2_ps = ps.tile([1, T], F32, tag="ps")
                nc.tensor.matmul(ms2_ps[:], mv[:], r32(f2[:]), start=True, stop=True)
                mu2_sb = wka.tile([1, T], F32, tag="msb")
                nc.scalar.copy(mu2_sb[:], mu2_ps[:])
                ms2_sb = wka.tile([1, T], F32, tag="ssb")
                nc.scalar.copy(ms2_sb[:], ms2_ps[:])
                nc.sync.dma_start(st2_dram[j:j + 1, :], mu2_sb[:])
                nc.sync.dma_start(st2_dram[16 + j:17 + j, :], ms2_sb[:])

            stats_finish(st2_dram, r2_dram, q2_dram)

            # ================= final phase: LN2 + MLP (gelu table) =========
            for j in range(NCH):
                sl = slice(j * T, (j + 1) * T)
                rft2 = wka.tile([1, T], F32, tag="rft")
                nc.sync.dma_start(r32(rft2[:]), r32(r2_dram[0:1, sl]))
                qft2 = wka.tile([1, T], F32, tag="qft")
                nc.sync.dma_start(r32(qft2[:]), r32(q2_dram[0:1, sl]))
                rb2 = ps.tile([C, T], F32, tag="ps")
                nc.tensor.matmul(rb2[:], ones1[:], r32(rft2[:]), start=True, stop=True)
                fr = wka.tile([C, T], F32, tag="fr")
                nc.vector.tensor_mul(r32(fr[:]), fbuf[:, sl], rb2[:])
                gtiles = []
                for h in range(2):
                    gp = ps.tile([2 * C, T], F32, tag="ps")
                    nc.tensor.matmul(gp[:], r32(Wfc1T[:, h * 2 * C:(h + 1) * 2 * C]),
                                     r32(fr[:]), start=True, stop=False,
                                     skip_group_check=True)
                    nc.tensor.matmul(gp[:], r32(w1f[:, h * 2 * C:(h + 1) * 2 * C]),
                                     r32(qft2[:]), start=False, stop=True,
                                     skip_group_check=True)
                    g = wka.tile([2 * C, T], F32, tag="g")
                    nc.scalar.activation(r32(g[:]), gp[:], AF.Gelu, bias=bfc1[:, h:h + 1])
                    gtiles.append(g)
                f2p = ps.tile([C, T], F32, tag="ps")
                nc.tensor.matmul(f2p[:], r32(Wfc2a[:]), r32(gtiles[0][:]),
                                 start=True, stop=False, skip_group_check=True)
                nc.tensor.matmul(f2p[:], r32(Wfc2b[:]), r32(gtiles[1][:]),
                                 start=False, stop=True, skip_group_check=True)
                outf = wka.tile([C, T], F32, tag="outf")
                nc.vector.tensor_add(outf[:], fbuf[:, sl], f2p[:])
                nc.sync.dma_start(out_d[:, sl], outf[:])

    nc.compile()
    return nc


def _get_program():
    if "nc" not in _PROG:
        _PROG["nc"] = _build_program()
    return _PROG["nc"]


# ----------------------------------------------------------------------------
# host weight preprocessing per direction
# ----------------------------------------------------------------------------

def _bf16_dtype():
    try:
        import ml_dtypes
        return ml_dtypes.bfloat16
    except ImportError:
        import jax.numpy as jnp
        return jnp.bfloat16


def _prep_weights(li, inputs):
    f32 = np.float32
    in_w = np.asarray(inputs["in_proj_w"][li], np.float64)
    nw = np.asarray(inputs["norm_w"][li], np.float64)
    nb = np.asarray(inputs["norm_b"][li], np.float64)
    W2 = in_w * nw[None, :]
    bz_full = in_w @ nb
    b_h, b_z = bz_full[:D_INNER], bz_full[D_INNER:]
    cw = np.asarray(inputs["conv_w"][li], np.float64)
    cb = np.asarray(inputs["conv_b"][li], np.float64)
    cbtot = cb + b_h * cw.sum(1)
    diag = np.zeros((D_INNER, 4, D_INNER), np.float64)
    kk = np.arange(D_INNER)
    for k in range(4):
        diag[kk, k, kk] = cw[:, k]
    xp = np.asarray(inputs["x_proj_w"][li], np.float64)
    fc1 = np.asarray(inputs["fc1_w"], np.float64)
    fw = np.asarray(inputs["fnorm_w"], np.float64)
    fb = np.asarray(inputs["fnorm_b"], np.float64)
    fc1p = fc1 * fw[None, :]
    bfc1 = fc1 @ fb
    fc2 = np.asarray(inputs["fc2_w"], np.float64)
    return {
        "W2T": W2.T.astype(f32),
        "w1n": (-W2.sum(1))[None, :].astype(f32),
        "diag": diag.reshape(D_INNER, 4 * D_INNER).astype(f32),
        "cbn": (-cbtot)[:, None].astype(f32),
        "cbp": cbtot[:, None].astype(f32),
        "bzn": (-b_z)[:, None].astype(f32),
        "bzp": b_z[:, None].astype(f32),
        "WxpA": xp[:DT_RANK].T.astype(f32),
        "WxpB": xp[DT_RANK:].T.astype(f32),
        "WdtT": np.asarray(inputs["dt_proj_w"][li], np.float64).T.astype(f32),
        "dtb": np.asarray(inputs["dt_proj_b"][li], f32)[:, None],
        "Aneg": (-np.exp(np.asarray(inputs["A_log"][li], np.float64))).astype(f32),
        "Dsk": np.asarray(inputs["D_skip"][li], f32)[:, None],
        "WoutT": np.asarray(inputs["out_proj_w"][li], np.float64).T.astype(f32),
        "Wfc1T": fc1p.T.astype(f32),
        "w1f": (-fc1p.sum(1))[None, :].astype(f32),
        "bfc1": bfc1.reshape(2, 128).T.astype(f32),
        "Wfc2a": fc2.T[:2 * C].astype(f32),
        "Wfc2b": fc2.T[2 * C:].astype(f32),
        "mv": np.full((C, 1), 1.0 / C, f32),
        "ones1": np.ones((1, C), f32),
        "epsb": np.full((16, 1), EPS, f32),
        "I128": np.eye(D_INNER).astype(_bf16_dtype()),
    }


def _reference_np(**inputs):
    """Pure-numpy fallback replica of the reference (slow, exact)."""
    i = int(inputs["src_number"]) % 4
    s = int(inputs["step_size"])
    feats, ori_h, ori_w = _rcds_np(np.asarray(inputs["ref_feat"], np.float32),
                                   np.asarray(inputs["src_feat"], np.float32), s, i)
    Bb, K, Cc, Ll = feats.shape
    f = feats.astype(np.float64)
    outs = np.empty_like(f)
    for d in range(4):
        li = d
        x = f[:, d].transpose(0, 2, 1)  # (B,L,C)
        mu = x.mean(-1, keepdims=True)
        var = ((x - mu) ** 2).mean(-1, keepdims=True)
        h = (x - mu) / np.sqrt(var + EPS) * np.asarray(inputs["norm_w"][li]) \
            + np.asarray(inputs["norm_b"][li])
        xz = h @ np.asarray(inputs["in_proj_w"][li]).T
        xh, z = xz[..., :D_INNER], xz[..., D_INNER:]
        xpd = np.pad(xh.transpose(0, 2, 1), ((0, 0), (0, 0), (3, 0)))
        cw = np.asarray(inputs["conv_w"][li])
        xc = sum(cw[:, k:k + 1] * xpd[:, :, k:k + Ll] for k in range(4))
        xc = xc + np.asarray(inputs["conv_b"][li])[None, :, None]
        xh = (xc / (1 + np.exp(-xc))).transpose(0, 2, 1)
        dbl = xh @ np.asarray(inputs["x_proj_w"][li]).T
        dtv = dbl[..., :DT_RANK]
        Bm = dbl[..., DT_RANK:DT_RANK + D_STATE]
        Cm = dbl[..., DT_RANK + D_STATE:]
        dtp = dtv @ np.asarray(inputs["dt_proj_w"][li]).T + np.asarray(inputs["dt_proj_b"][li])
        dtv = np.logaddexp(0, dtp)
        A = -np.exp(np.asarray(inputs["A_log"][li], np.float64))
        dA = np.exp(dtv[..., None] * A)
        dBu = (dtv * xh)[..., None] * Bm[:, :, None, :]
        hst = np.zeros((Bb, D_INNER, D_STATE))
        ys = np.empty((Bb, Ll, D_INNER))
        for t in range(Ll):
            hst = dA[:, t] * hst + dBu[:, t]
            ys[:, t] = np.einsum("bdn,bn->bd", hst, Cm[:, t])
        ys = ys + xh * np.asarray(inputs["D_skip"][li])
        ys = ys * (z / (1 + np.exp(-z)))
        o = ys @ np.asarray(inputs["out_proj_w"][li]).T
        outs[:, d] = (x + o).transpose(0, 2, 1)
    x = outs.transpose(0, 1, 3, 2)  # (B,4,L,C)
    mu = x.mean(-1, keepdims=True)
    var = ((x - mu) ** 2).mean(-1, keepdims=True)
    h = (x - mu) / np.sqrt(var + EPS) * np.asarray(inputs["fnorm_w"]) \
        + np.asarray(inputs["fnorm_b"])
    from scipy.special import erf
    g = h @ np.asarray(inputs["fc1_w"]).T
    g = 0.5 * g * (1 + erf(g / np.sqrt(2)))
    x = x + g @ np.asarray(inputs["fc2_w"]).T
    d0, d1 = _merge_np(x.transpose(0, 1, 3, 2).astype(np.float32), ori_h, ori_w, s, i)
    return d0.astype(np.float32), d1.astype(np.float32)


def kernel(**inputs):
    s = int(inputs["step_size"])
    i = int(inputs["src_number"]) % 4
    ref_feat = np.asarray(inputs["ref_feat"], np.float32)
    src_feat = np.asarray(inputs["src_feat"], np.float32)

    if s != 2 or ref_feat.shape != (B, C, HIMG, WIMG):
        return _reference_np(**inputs)

    feats, ori_h, ori_w = _rcds_np(ref_feat, src_feat, s, i)  # (B,4,C,L)

    wmaps = [_prep_weights(d, inputs) for d in range(4)]
    in_maps = []
    for core in range(8):
        b, d = core // 4, core % 4
        m = dict(wmaps[d])
        m["x"] = np.ascontiguousarray(feats[b, d])
        in_maps.append(m)

    from concourse.bass_utils import run_bass_kernel_spmd
    nc = _get_program()
    res = run_bass_kernel_spmd(nc, in_maps, list(range(8)))

    ys = np.empty((B, 4, C, L), np.float32)
    for core in range(8):
        b, d = core // 4, core % 4
        ys[b, d] = res.results[core]["out"]

    d0, d1 = _merge_np(ys, ori_h, ori_w, s, i)
    return d0, d1



# revision 12
# speedup vs baseline: 1.3782x; 1.3782x over previous
"""Trainium2 Bass kernel for nn_DM_35210141892754 (4-direction VMamba block).

Sharding: 8 cores = B(2) x directions(4); each core processes one gathered
sequence (C=64, L=8192) end-to-end. Gather/scatter (reference _rcds/_merge)
run on host with numpy strided ops; all FLOPs run on device.
"""

import sys
import math

sys.path.insert(0, "/opt/trn_rl_repo")

import numpy as np

B, C, HIMG, WIMG = 2, 64, 128, 128
DEPTH = 4
D_STATE, D_CONV, EXPAND = 16, 4, 2
D_INNER = EXPAND * C  # 128
DT_RANK = math.ceil(C / 16)  # 4
L = 8192  # sequence length for step_size=2
T = 512  # device chunk size
NCH = L // T
EPS = 1e-5

OFF = {0: ((1, 0), (0, 0), (0, 1), (1, 1)),
       1: ((0, 0), (1, 0), (1, 1), (0, 1)),
       2: ((0, 1), (1, 1), (1, 0), (0, 0)),
       3: ((1, 1), (0, 1), (0, 0), (1, 0))}

_PROG = {}


# ----------------------------------------------------------------------------
# host-side gather (reference _rcds) and scatter (reference _merge), numpy
# ----------------------------------------------------------------------------

def _rcds_np(f0, f1, s, i):
    fr = np.concatenate([f0, f1], axis=3)
    fl = np.concatenate([f1, f0], axis=3)
    fb = np.concatenate([f0, f1], axis=2)
    ft = np.concatenate([f1, f0], axis=2)
    Bb, Cc = fr.shape[:2]
    r, l, b, t = OFF[i]
    y0 = fr[:, :, r[0]::s, r[1]::s].transpose(0, 1, 3, 2).reshape(Bb, Cc, -1)
    y1 = fl[:, :, l[0]::s, l[1]::s].transpose(0, 1, 3, 2).reshape(Bb, Cc, -1)[:, :, ::-1]
    y2 = fb[:, :, b[0]::s, b[1]::s].reshape(Bb, Cc, -1)
    y3 = ft[:, :, t[0]::s, t[1]::s].reshape(Bb, Cc, -1)[:, :, ::-1]
    feats = np.stack([y0, y1, y2, y3], axis=1)  # (B, 4, C, L)
    return np.ascontiguousarray(feats), fr.shape[2], fb.shape[3]


def _merge_np(ys, ori_h, ori_w, s, i):
    # ys: (B, 4, C, L)
    Bb, K, Cc, Ll = ys.shape
    Hh = -(-ori_h // s)
    Ww = -(-ori_w // s)
    nh, nw = Hh * s, Ww * s
    r, l, b, t = OFF[i]
    y2wr = np.zeros((Bb, Cc, nh, 2 * nw), ys.dtype)
    y2wl = np.zeros((Bb, Cc, nh, 2 * nw), ys.dtype)
    y2hb = np.zeros((Bb, Cc, 2 * nh, nw), ys.dtype)
    y2ht = np.zeros((Bb, Cc, 2 * nh, nw), ys.dtype)
    y2wr[:, :, r[0]::s, r[1]::s] = ys[:, 0].reshape(Bb, Cc, 2 * Ww, Hh).transpose(0, 1, 3, 2)
    y2wl[:, :, l[0]::s, l[1]::s] = ys[:, 1][:, :, ::-1].reshape(Bb, Cc, 2 * Ww, Hh).transpose(0, 1, 3, 2)
    y2hb[:, :, b[0]::s, b[1]::s] = ys[:, 2].reshape(Bb, Cc, 2 * Hh, Ww)
    y2ht[:, :, t[0]::s, t[1]::s] = ys[:, 3][:, :, ::-1].reshape(Bb, Cc, 2 * Hh, Ww)
    if ori_h != nh or ori_w != nw:
        y2wr = y2wr[:, :, :ori_h, :ori_w]
        y2wl = y2wl[:, :, :ori_h, :ori_w]
        y2ht = y2ht[:, :, :ori_h, :ori_w]
        y2hb = y2hb[:, :, :ori_h, :ori_w]
    d0r, d1r = np.split(y2wr, 2, axis=3)
    d1l, d0l = np.split(y2wl, 2, axis=3)
    d0b, d1b = np.split(y2hb, 2, axis=2)
    d1t, d0t = np.split(y2ht, 2, axis=2)
    return d0r + d0l + d0b + d0t, d1r + d1l + d1b + d1t


# ----------------------------------------------------------------------------
# device program
# ----------------------------------------------------------------------------

def _build_program():
    import concourse.bacc as bacc
    import concourse.mybir as mybir
    import concourse.tile as tile

    dt_ = mybir.dt
    F32, BF16, F32R = dt_.float32, dt_.bfloat16, dt_.float32r
    AF = mybir.ActivationFunctionType
    OP = mybir.AluOpType

    def r32(ap):
        return ap.bitcast(F32R)

    nc = bacc.Bacc("TRN2", target_bir_lowering=False, debug=False)

    def din(name, shape, d=F32):
        return nc.dram_tensor(name, shape, d, kind="ExternalInput")

    x_d = din("x", [C, L])
    W2T_d = din("W2T", [C, 2 * D_INNER])
    w1n_d = din("w1n", [1, 2 * D_INNER])
    diag_d = din("diag", [D_INNER, 4 * D_INNER])
    cbn_d = din("cbn", [D_INNER, 1])
    cbp_d = din("cbp", [D_INNER, 1])
    bzn_d = din("bzn", [D_INNER, 1])
    bzp_d = din("bzp", [D_INNER, 1])
    WxpA_d = din("WxpA", [D_INNER, DT_RANK])
    WxpB_d = din("WxpB", [D_INNER, 2 * D_STATE])
    WdtT_d = din("WdtT", [DT_RANK, D_INNER])
    dtb_d = din("dtb", [D_INNER, 1])
    Aneg_d = din("Aneg", [D_INNER, D_STATE])
    Dsk_d = din("Dsk", [D_INNER, 1])
    WoutT_d = din("WoutT", [D_INNER, C])
    Wfc1T_d = din("Wfc1T", [C, 4 * C])
    w1f_d = din("w1f", [1, 4 * C])
    bfc1_d = din("bfc1", [4 * C // 2, 2])
    Wfc2a_d = din("Wfc2a", [2 * C, C])
    Wfc2b_d = din("Wfc2b", [2 * C, C])
    mv_d = din("mv", [C, 1])
    mvb_d = din("mvb", [C, 1], dt_.bfloat16)
    ones1_d = din("ones1", [1, C])
    epsb_d = din("epsb", [16, 1])
    I128_d = din("I128", [D_INNER, D_INNER], dt_.bfloat16)

    out_d = nc.dram_tensor("out", [C, L], F32, kind="ExternalOutput")

    NS = D_STATE

    with tile.TileContext(nc) as tc:
        with tc.tile_pool(name="pers", bufs=1) as pers, \
             tc.tile_pool(name="wka", bufs=2) as wka, \
             tc.tile_pool(name="wkb", bufs=3) as wkb, \
             tc.tile_pool(name="wk2", bufs=1) as wk2, \
             tc.tile_pool(name="hp", bufs=2) as hp, \
             tc.tile_pool(name="cvp", bufs=2) as cvp, \
             tc.tile_pool(name="bgp", bufs=2) as bgp, \
             tc.tile_pool(name="dap", bufs=2) as dap, \
             tc.tile_pool(name="dbp", bufs=2) as dbp, \
             tc.tile_pool(name="ps", bufs=5, space="PSUM") as ps, \
             tc.tile_pool(name="pacc", bufs=2, space="PSUM") as pacc, \
             tc.tile_pool(name="dr", bufs=1, space="DRAM") as dr:

            def ld(dram, shape, d=F32, tag=None, rr=False):
                t_ = pers.tile(shape, F32R if rr else d, tag=tag)
                if rr:
                    nc.sync.dma_start(t_[:], r32(dram[:]))
                else:
                    nc.sync.dma_start(t_[:], dram[:])
                return t_

            W2T = ld(W2T_d, [C, 2 * D_INNER], tag="W2T", rr=True)
            w1n = ld(w1n_d, [1, 2 * D_INNER], tag="w1n", rr=True)
            diag = ld(diag_d, [D_INNER, 4 * D_INNER], tag="diag", rr=True)
            cbn = ld(cbn_d, [D_INNER, 1], tag="cbn")
            cbp = ld(cbp_d, [D_INNER, 1], tag="cbp")
            bzn = ld(bzn_d, [D_INNER, 1], tag="bzn")
            bzp = ld(bzp_d, [D_INNER, 1], tag="bzp")
            WxpA = ld(WxpA_d, [D_INNER, DT_RANK], tag="WxpA", rr=True)
            WxpB = ld(WxpB_d, [D_INNER, 2 * D_STATE], tag="WxpB", rr=True)
            WdtT = ld(WdtT_d, [DT_RANK, D_INNER], tag="WdtT", rr=True)
            dtb = ld(dtb_d, [D_INNER, 1], tag="dtb")
            Aneg = ld(Aneg_d, [D_INNER, D_STATE], tag="Aneg")
            Dsk = ld(Dsk_d, [D_INNER, 1], tag="Dsk")
            WoutT = ld(WoutT_d, [D_INNER, C], tag="WoutT", rr=True)
            Wfc1T = ld(Wfc1T_d, [C, 4 * C], tag="Wfc1T", rr=True)
            w1f = ld(w1f_d, [1, 4 * C], tag="w1f", rr=True)
            bfc1 = ld(bfc1_d, [4 * C // 2, 2], tag="bfc1")
            Wfc2a = ld(Wfc2a_d, [2 * C, C], tag="Wfc2a", rr=True)
            Wfc2b = ld(Wfc2b_d, [2 * C, C], tag="Wfc2b", rr=True)
            mv = ld(mv_d, [C, 1], tag="mv", rr=True)
            mvb = ld(mvb_d, [C, 1], BF16, tag="mvb")
            ones1 = ld(ones1_d, [1, C], tag="ones1", rr=True)
            epsb = ld(epsb_d, [16, 1], tag="epsb")
            I128 = ld(I128_d, [D_INNER, D_INNER], BF16, tag="I128")

            fbuf = pers.tile([C, L], BF16, tag="fbuf")

            st1_dram = dr.tile([32, T], F32, tag="st1d")
            r1_dram = dr.tile([1, L], F32, tag="r1d")
            q1_dram = dr.tile([1, L], F32, tag="q1d")
            st2_dram = dr.tile([32, T], F32, tag="st2d")
            r2_dram = dr.tile([1, L], F32, tag="r2d")
            q2_dram = dr.tile([1, L], F32, tag="q2d")
            # chunk-major so broadcast-source rows are contiguous in DRAM
            bc_dram = dr.tile([NCH * 2 * NS, T], BF16, tag="bcd")

            # ================= phase 0: LN1 stats =================
            for j in range(NCH):
                sl = slice(j * T, (j + 1) * T)
                xj = wka.tile([C, T], F32, tag="xj")
                nc.sync.dma_start(r32(xj[:]), r32(x_d[:, sl]))
                x2 = wka.tile([C, T], F32, tag="f2")
                nc.scalar.activation(r32(x2[:]), xj[:], AF.Square)
                mu_ps = ps.tile([1, T], F32, tag="ps")
                nc.tensor.matmul(mu_ps[:], mv[:], r32(xj[:]), start=True, stop=True)
                ms_ps = ps.tile([1, T], F32, tag="ps")
                nc.tensor.matmul(ms_ps[:], mv[:], r32(x2[:]), start=True, stop=True)
                mu_sb = wka.tile([1, T], F32, tag="msb")
                nc.scalar.copy(mu_sb[:], mu_ps[:])
                ms_sb = wka.tile([1, T], F32, tag="ssb")
                nc.scalar.copy(ms_sb[:], ms_ps[:])
                nc.sync.dma_start(st1_dram[j:j + 1, :], mu_sb[:])
                nc.sync.dma_start(st1_dram[16 + j:17 + j, :], ms_sb[:])

            def stats_finish(st_dram, rf_dram, qf_dram):
                mu_all = wk2.tile([NCH, T], F32, tag="sa")
                nc.sync.dma_start(mu_all[:], st_dram[0:16, :])
                ms_all = wk2.tile([NCH, T], F32, tag="sb")
                nc.sync.dma_start(ms_all[:], st_dram[16:32, :])
                t1 = wk2.tile([NCH, T], F32, tag="sc")
                nc.scalar.activation(t1[:], mu_all[:], AF.Square)
                t2 = wk2.tile([NCH, T], F32, tag="sd")
                nc.vector.tensor_sub(t2[:], ms_all[:], t1[:])
                t3 = wk2.tile([NCH, T], F32, tag="sc")
                nc.scalar.activation(t3[:], t2[:], AF.Ln, bias=epsb[:])
                rstd = wk2.tile([NCH, T], F32, tag="sd")
                nc.scalar.activation(rstd[:], t3[:], AF.Exp, bias=0.0, scale=-0.5)
                mq = wk2.tile([NCH, T], F32, tag="sc")
                nc.vector.tensor_mul(mq[:], mu_all[:], rstd[:])
                nc.sync.dma_start(rf_dram[:].rearrange("a (c t) -> (a c) t", t=T), rstd[:])
                nc.sync.dma_start(qf_dram[:].rearrange("a (c t) -> (a c) t", t=T), mq[:])

            stats_finish(st1_dram, r1_dram, q1_dram)

            # ================= steady phase =================
            G = 4   # states per tile group
            GD = 2  # states per broadcast DMA (descriptor-size tradeoff)
            NG = NS // G
            prev_h = None
            prev_cv = None
            for j in range(NCH):
                sl = slice(j * T, (j + 1) * T)
                bcr = j * 2 * NS  # chunk-major row base in bc_dram
                xj = wka.tile([C, T], F32, tag="xj")
                nc.sync.dma_start(xj[:], x_d[:, sl])
                rft = wka.tile([1, T], F32, tag="rft")
                nc.sync.dma_start(r32(rft[:]), r32(r1_dram[0:1, sl]))
                qft = wka.tile([1, T], F32, tag="qft")
                nc.sync.dma_start(r32(qft[:]), r32(q1_dram[0:1, sl]))

                rb = ps.tile([C, T], F32, tag="ps")
                nc.tensor.matmul(rb[:], ones1[:], r32(rft[:]), start=True, stop=True)
                xr = wka.tile([C, T], F32, tag="xr")
                nc.vector.tensor_mul(r32(xr[:]), xj[:], rb[:])

                xzh = ps.tile([D_INNER, T], F32, tag="ps")
                nc.tensor.matmul(xzh[:], r32(W2T[:, 0:D_INNER]), r32(xr[:]),
                                 start=True, stop=False, skip_group_check=True)
                nc.tensor.matmul(xzh[:], r32(w1n[:, 0:D_INNER]), r32(qft[:]),
                                 start=False, stop=True, skip_group_check=True)
                xzz = ps.tile([D_INNER, T], F32, tag="ps")
                nc.tensor.matmul(xzz[:], r32(W2T[:, D_INNER:2 * D_INNER]), r32(xr[:]),
                                 start=True, stop=False, skip_group_check=True)
                nc.tensor.matmul(xzz[:], r32(w1n[:, D_INNER:2 * D_INNER]), r32(qft[:]),
                                 start=False, stop=True, skip_group_check=True)

                cv = cvp.tile([D_INNER, T + 3], F32, tag="cv")
                if j == 0:
                    nc.vector.memset(cv[:, 0:3], 0.0)
                else:
                    nc.vector.tensor_copy(r32(cv[:, 0:3]), r32(prev_cv[:, T:T + 3]))
                nc.scalar.activation(r32(cv[:, 3:T + 3]), xzh[:], AF.Copy)
                cps = ps.tile([D_INNER, T], F32, tag="ps")
                for k in range(4):
                    nc.tensor.matmul(cps[:], r32(diag[:, k * D_INNER:(k + 1) * D_INNER]),
                                     r32(cv[:, k:k + T]),
                                     start=(k == 0), stop=(k == 3), skip_group_check=True)
                prev_cv = cv

                # both silus via sigmoid = exp(-ln(1+e^-u)); Exp/Ln ops clustered
                # to minimize activation-table switches
                ez = wka.tile([D_INNER, T], F32, tag="tmpa")
                nc.scalar.activation(ez[:], xzz[:], AF.Exp, bias=bzn[:], scale=-1.0)
                ec = wka.tile([D_INNER, T], F32, tag="tmpc")
                nc.scalar.activation(ec[:], cps[:], AF.Exp, bias=cbn[:], scale=-1.0)
                nc.vector.tensor_scalar_add(ez[:], ez[:], 1.0)
                nc.vector.tensor_scalar_add(ec[:], ec[:], 1.0)
                lz = wka.tile([D_INNER, T], F32, tag="tmpb")
                nc.scalar.activation(lz[:], ez[:], AF.Ln)
                lc = wka.tile([D_INNER, T], F32, tag="tmpd")
                nc.scalar.activation(lc[:], ec[:], AF.Ln)
                gc = wka.tile([D_INNER, T], F32, tag="tmpc")
                nc.scalar.activation(gc[:], lc[:], AF.Exp, bias=0.0, scale=-1.0)
                gz = wka.tile([D_INNER, T], F32, tag="tmpa")
                nc.scalar.activation(gz[:], lz[:], AF.Exp, bias=0.0, scale=-1.0)
                xh = wka.tile([D_INNER, T], F32, tag="xh")
                nc.vector.scalar_tensor_tensor(r32(xh[:]), cps[:], cbp[:], gc[:],
                                               OP.add, OP.mult)
                sz = wka.tile([D_INNER, T], F32, tag="sz")
                nc.vector.scalar_tensor_tensor(sz[:], xzz[:], bzp[:], gz[:], OP.add, OP.mult)

                dblA = ps.tile([DT_RANK, T], F32, tag="ps")
                nc.tensor.matmul(dblA[:], r32(WxpA[:]), r32(xh[:]), start=True, stop=True)
                dblB = ps.tile([2 * NS, T], F32, tag="ps")
                nc.tensor.matmul(dblB[:], r32(WxpB[:]), r32(xh[:]), start=True, stop=True)
                dtin = wkb.tile([DT_RANK, T], F32, tag="dtin")
                nc.scalar.activation(r32(dtin[:]), dblA[:], AF.Copy)
                bc = wkb.tile([2 * NS, T], BF16, tag="bc")
                nc.scalar.copy(bc[:], dblB[:])
                nc.sync.dma_start(bc_dram[bcr:bcr + 2 * NS, :], bc[:])

                # broadcast B/C state-rows to all partitions; contiguous 2-row
                # groups in chunk-major bc_dram -> 1 descriptor per partition
                Bgs, Cgs = [], []
                for g in range(NG):
                    Bg = bgp.tile([D_INNER, G, T], BF16, tag="Bg")
                    for h in range(G // GD):
                        r0 = bcr + g * G + h * GD
                        nc.sync.dma_start(
                            Bg[:, h * GD:(h + 1) * GD, :],
                            bc_dram[r0:r0 + GD, :][None]
                            .broadcast_to([D_INNER, GD, T]))
                    Bgs.append(Bg)
                    Cg = bgp.tile([D_INNER, G, T], BF16, tag="Cg")
                    for h in range(G // GD):
                        r0 = bcr + NS + g * G + h * GD
                        nc.sync.dma_start(
                            Cg[:, h * GD:(h + 1) * GD, :],
                            bc_dram[r0:r0 + GD, :][None]
                            .broadcast_to([D_INNER, GD, T]))
                    Cgs.append(Cg)

                dtp = ps.tile([D_INNER, T], F32, tag="ps")
                nc.tensor.matmul(dtp[:], r32(WdtT[:]), r32(dtin[:]), start=True, stop=True)
                esp = wka.tile([D_INNER, T], F32, tag="tmpa")
                nc.scalar.activation(esp[:], dtp[:], AF.Exp, bias=dtb[:], scale=1.0)
                nc.vector.tensor_scalar_add(esp[:], esp[:], 1.0)
                dt = wka.tile([D_INNER, T], F32, tag="dt")
                nc.scalar.activation(dt[:], esp[:], AF.Ln)

                w = wkb.tile([D_INNER, T], BF16, tag="w")
                nc.vector.tensor_mul(w[:], dt[:], xh[:])

                cur_h = hp.tile([D_INNER, NS, T], BF16, tag="h")
                yacc = pacc.tile([D_INNER, T], F32, tag="yacc")
                for g in range(NG):
                    dAg = dap.tile([D_INNER, G, T], F32, tag="dAg")
                    for k in range(G):
                        n = g * G + k
                        nc.scalar.activation(dAg[:, k, :], dt[:], AF.Exp, bias=0.0,
                                             scale=Aneg[:, n:n + 1])
                    dBg = dbp.tile([D_INNER, G, T], BF16, tag="dBg")
                    nc.vector.tensor_tensor(dBg[:], w[:][:, None].broadcast_to(
                        [D_INNER, G, T]), Bgs[g][:], OP.mult)
                    for k in range(G):
                        n = g * G + k
                        init = 0.0 if j == 0 else prev_h[:, n, T - 1:T]
                        nc.vector.tensor_tensor_scan(cur_h[:, n, :], dAg[:, k, :],
                                                     dBg[:, k, :], init,
                                                     OP.mult, OP.add)
                    Pg = dbp.tile([D_INNER, G, T], BF16, tag="Pg")
                    if g == 0:
                        nc.gpsimd.tensor_tensor(Pg[:], cur_h[:, g * G:(g + 1) * G, :],
                                                Cgs[g][:], OP.mult)
                    else:
                        nc.vector.tensor_tensor(Pg[:], cur_h[:, g * G:(g + 1) * G, :],
                                                Cgs[g][:], OP.mult)
                    for k in range(G):
                        n = g * G + k
                        nc.tensor.matmul(yacc[:], I128[:], Pg[:, k, :], start=(n == 0),
                                         stop=(n == NS - 1), skip_group_check=True)
                prev_h = cur_h

                y = wka.tile([D_INNER, T], F32, tag="y")
                nc.vector.scalar_tensor_tensor(y[:], xh[:], Dsk[:], yacc[:],
                                               OP.mult, OP.add)
                gated = wka.tile([D_INNER, T], F32, tag="gated")
                nc.vector.tensor_mul(r32(gated[:]), y[:], sz[:])
                opj = ps.tile([C, T], F32, tag="ps")
                nc.tensor.matmul(opj[:], r32(WoutT[:]), r32(gated[:]), start=True, stop=True)
                nc.vector.tensor_add(fbuf[:, sl], xj[:], opj[:])

                f2 = wka.tile([C, T], F32, tag="f2")
                nc.scalar.activation(r32(f2[:]), fbuf[:, sl], AF.Square)
                mu2_ps = ps.tile([1, T], F32, tag="ps")
                nc.tensor.matmul(mu2_ps[:], mvb[:], fbuf[:, sl], start=True, stop=True)
                ms2_ps = ps.tile([1, T], F32, tag="ps")
                nc.tensor.matmul(ms2_ps[:], mv[:], r32(f2[:]), start=True, stop=True)
                mu2_sb = wka.tile([1, T], F32, tag="msb")
                nc.scalar.copy(mu2_sb[:], mu2_ps[:])
                ms2_sb = wka.tile([1, T], F32, tag="ssb")
                nc.scalar.copy(ms2_sb[:], ms2_ps[:])
                nc.sync.dma_start(st2_dram[j:j + 1, :], mu2_sb[:])
                nc.sync.dma_start(st2_dram[16 + j:17 + j, :], ms2_sb[:])

            stats_finish(st2_dram, r2_dram, q2_dram)

            # ================= final phase: LN2 + MLP (gelu table) =========
            for j in range(NCH):
                sl = slice(j * T, (j + 1) * T)
                rft2 = wka.tile([1, T], F32, tag="rft")
                nc.sync.dma_start(r32(rft2[:]), r32(r2_dram[0:1, sl]))
                qft2 = wka.tile([1, T], F32, tag="qft")
                nc.sync.dma_start(r32(qft2[:]), r32(q2_dram[0:1, sl]))
                rb2 = ps.tile([C, T], F32, tag="ps")
                nc.tensor.matmul(rb2[:], ones1[:], r32(rft2[:]), start=True, stop=True)
                fr = wka.tile([C, T], F32, tag="fr")
                nc.vector.tensor_mul(r32(fr[:]), fbuf[:, sl], rb2[:])
                gtiles = []
                for h in range(2):
                    gp = ps.tile([2 * C, T], F32, tag="ps")
                    nc.tensor.matmul(gp[:], r32(Wfc1T[:, h * 2 * C:(h + 1) * 2 * C]),
                                     r32(fr[:]), start=True, stop=False,
                                     skip_group_check=True)
                    nc.tensor.matmul(gp[:], r32(w1f[:, h * 2 * C:(h + 1) * 2 * C]),
                                     r32(qft2[:]), start=False, stop=True,
                                     skip_group_check=True)
                    g = wka.tile([2 * C, T], F32, tag="g")
                    nc.scalar.activation(r32(g[:]), gp[:], AF.Gelu, bias=bfc1[:, h:h + 1])
                    gtiles.append(g)
                f2p = ps.tile([C, T], F32, tag="ps")
                nc.tensor.matmul(f2p[:], r32(Wfc2a[:]), r32(gtiles[0][:]),
                                 start=True, stop=False, skip_group_check=True)
                nc.tensor.matmul(f2p[:], r32(Wfc2b[:]), r32(gtiles[1][:]),
                                 start=False, stop=True, skip_group_check=True)
                outf = wka.tile([C, T], F32, tag="outf")
                nc.vector.tensor_add(outf[:], fbuf[:, sl], f2p[:])
                nc.sync.dma_start(out_d[:, sl], outf[:])

    nc.compile()
    return nc


def _get_program():
    if "nc" not in _PROG:
        _PROG["nc"] = _build_program()
    return _PROG["nc"]


# ----------------------------------------------------------------------------
# host weight preprocessing per direction
# ----------------------------------------------------------------------------

def _bf16_dtype():
    try:
        import ml_dtypes
        return ml_dtypes.bfloat16
    except ImportError:
        import jax.numpy as jnp
        return jnp.bfloat16


def _prep_weights(li, inputs):
    f32 = np.float32
    in_w = np.asarray(inputs["in_proj_w"][li], np.float64)
    nw = np.asarray(inputs["norm_w"][li], np.float64)
    nb = np.asarray(inputs["norm_b"][li], np.float64)
    W2 = in_w * nw[None, :]
    bz_full = in_w @ nb
    b_h, b_z = bz_full[:D_INNER], bz_full[D_INNER:]
    cw = np.asarray(inputs["conv_w"][li], np.float64)
    cb = np.asarray(inputs["conv_b"][li], np.float64)
    cbtot = cb + b_h * cw.sum(1)
    diag = np.zeros((D_INNER, 4, D_INNER), np.float64)
    kk = np.arange(D_INNER)
    for k in range(4):
        diag[kk, k, kk] = cw[:, k]
    xp = np.asarray(inputs["x_proj_w"][li], np.float64)
    fc1 = np.asarray(inputs["fc1_w"], np.float64)
    fw = np.asarray(inputs["fnorm_w"], np.float64)
    fb = np.asarray(inputs["fnorm_b"], np.float64)
    fc1p = fc1 * fw[None, :]
    bfc1 = fc1 @ fb
    fc2 = np.asarray(inputs["fc2_w"], np.float64)
    return {
        "W2T": W2.T.astype(f32),
        "w1n": (-W2.sum(1))[None, :].astype(f32),
        "diag": diag.reshape(D_INNER, 4 * D_INNER).astype(f32),
        "cbn": (-cbtot)[:, None].astype(f32),
        "cbp": cbtot[:, None].astype(f32),
        "bzn": (-b_z)[:, None].astype(f32),
        "bzp": b_z[:, None].astype(f32),
        "WxpA": xp[:DT_RANK].T.astype(f32),
        "WxpB": xp[DT_RANK:].T.astype(f32),
        "WdtT": np.asarray(inputs["dt_proj_w"][li], np.float64).T.astype(f32),
        "dtb": np.asarray(inputs["dt_proj_b"][li], f32)[:, None],
        "Aneg": (-np.exp(np.asarray(inputs["A_log"][li], np.float64))).astype(f32),
        "Dsk": np.asarray(inputs["D_skip"][li], f32)[:, None],
        "WoutT": np.asarray(inputs["out_proj_w"][li], np.float64).T.astype(f32),
        "Wfc1T": fc1p.T.astype(f32),
        "w1f": (-fc1p.sum(1))[None, :].astype(f32),
        "bfc1": bfc1.reshape(2, 128).T.astype(f32),
        "Wfc2a": fc2.T[:2 * C].astype(f32),
        "Wfc2b": fc2.T[2 * C:].astype(f32),
        "mv": np.full((C, 1), 1.0 / C, f32),
        "mvb": np.full((C, 1), 1.0 / C).astype(_bf16_dtype()),
        "ones1": np.ones((1, C), f32),
        "epsb": np.full((16, 1), EPS, f32),
        "I128": np.eye(D_INNER).astype(_bf16_dtype()),
    }


def _reference_np(**inputs):
    """Pure-numpy fallback replica of the reference (slow, exact)."""
    i = int(inputs["src_number"]) % 4
    s = int(inputs["step_size"])
    feats, ori_h, ori_w = _rcds_np(np.asarray(inputs["ref_feat"], np.float32),
                                   np.asarray(inputs["src_feat"], np.float32), s, i)
    Bb, K, Cc, Ll = feats.shape
    f = feats.astype(np.float64)
    outs = np.empty_like(f)
    for d in range(4):
        li = d
        x = f[:, d].transpose(0, 2, 1)  # (B,L,C)
        mu = x.mean(-1, keepdims=True)
        var = ((x - mu) ** 2).mean(-1, keepdims=True)
        h = (x - mu) / np.sqrt(var + EPS) * np.asarray(inputs["norm_w"][li]) \
            + np.asarray(inputs["norm_b"][li])
        xz = h @ np.asarray(inputs["in_proj_w"][li]).T
        xh, z = xz[..., :D_INNER], xz[..., D_INNER:]
        xpd = np.pad(xh.transpose(0, 2, 1), ((0, 0), (0, 0), (3, 0)))
        cw = np.asarray(inputs["conv_w"][li])
        xc = sum(cw[:, k:k + 1] * xpd[:, :, k:k + Ll] for k in range(4))
        xc = xc + np.asarray(inputs["conv_b"][li])[None, :, None]
        xh = (xc / (1 + np.exp(-xc))).transpose(0, 2, 1)
        dbl = xh @ np.asarray(inputs["x_proj_w"][li]).T
        dtv = dbl[..., :DT_RANK]
        Bm = dbl[..., DT_RANK:DT_RANK + D_STATE]
        Cm = dbl[..., DT_RANK + D_STATE:]
        dtp = dtv @ np.asarray(inputs["dt_proj_w"][li]).T + np.asarray(inputs["dt_proj_b"][li])
        dtv = np.logaddexp(0, dtp)
        A = -np.exp(np.asarray(inputs["A_log"][li], np.float64))
        dA = np.exp(dtv[..., None] * A)
        dBu = (dtv * xh)[..., None] * Bm[:, :, None, :]
        hst = np.zeros((Bb, D_INNER, D_STATE))
        ys = np.empty((Bb, Ll, D_INNER))
        for t in range(Ll):
            hst = dA[:, t] * hst + dBu[:, t]
            ys[:, t] = np.einsum("bdn,bn->bd", hst, Cm[:, t])
        ys = ys + xh * np.asarray(inputs["D_skip"][li])
        ys = ys * (z / (1 + np.exp(-z)))
        o = ys @ np.asarray(inputs["out_proj_w"][li]).T
        outs[:, d] = (x + o).transpose(0, 2, 1)
    x = outs.transpose(0, 1, 3, 2)  # (B,4,L,C)
    mu = x.mean(-1, keepdims=True)
    var = ((x - mu) ** 2).mean(-1, keepdims=True)
    h = (x - mu) / np.sqrt(var + EPS) * np.asarray(inputs["fnorm_w"]) \
        + np.asarray(inputs["fnorm_b"])
    from scipy.special import erf
    g = h @ np.asarray(inputs["fc1_w"]).T
    g = 0.5 * g * (1 + erf(g / np.sqrt(2)))
    x = x + g @ np.asarray(inputs["fc2_w"]).T
    d0, d1 = _merge_np(x.transpose(0, 1, 3, 2).astype(np.float32), ori_h, ori_w, s, i)
    return d0.astype(np.float32), d1.astype(np.float32)


def kernel(**inputs):
    s = int(inputs["step_size"])
    i = int(inputs["src_number"]) % 4
    ref_feat = np.asarray(inputs["ref_feat"], np.float32)
    src_feat = np.asarray(inputs["src_feat"], np.float32)

    if s != 2 or ref_feat.shape != (B, C, HIMG, WIMG):
        return _reference_np(**inputs)

    feats, ori_h, ori_w = _rcds_np(ref_feat, src_feat, s, i)  # (B,4,C,L)

    wmaps = [_prep_weights(d, inputs) for d in range(4)]
    in_maps = []
    for core in range(8):
        b, d = core // 4, core % 4
        m = dict(wmaps[d])
        m["x"] = np.ascontiguousarray(feats[b, d])
        in_maps.append(m)

    from concourse.bass_utils import run_bass_kernel_spmd
    nc = _get_program()
    res = run_bass_kernel_spmd(nc, in_maps, list(range(8)))

    ys = np.empty((B, 4, C, L), np.float32)
    for core in range(8):
        b, d = core // 4, core % 4
        ys[b, d] = res.results[core]["out"]

    d0, d1 = _merge_np(ys, ori_h, ori_w, s, i)
    return d0, d1



# revision 14
# speedup vs baseline: 1.6883x; 1.2250x over previous
"""Trainium2 Bass kernel for nn_DM_35210141892754 (4-direction VMamba block).

Sharding: 8 cores = B(2) x directions(4); each core processes one gathered
sequence (C=64, L=8192) end-to-end. Gather/scatter (reference _rcds/_merge)
run on host with numpy strided ops; all FLOPs run on device.
"""

import sys
import math

sys.path.insert(0, "/opt/trn_rl_repo")

import numpy as np

B, C, HIMG, WIMG = 2, 64, 128, 128
DEPTH = 4
D_STATE, D_CONV, EXPAND = 16, 4, 2
D_INNER = EXPAND * C  # 128
DT_RANK = math.ceil(C / 16)  # 4
L = 8192  # sequence length for step_size=2
T = 512  # device chunk size
NCH = L // T
EPS = 1e-5

OFF = {0: ((1, 0), (0, 0), (0, 1), (1, 1)),
       1: ((0, 0), (1, 0), (1, 1), (0, 1)),
       2: ((0, 1), (1, 1), (1, 0), (0, 0)),
       3: ((1, 1), (0, 1), (0, 0), (1, 0))}

_PROG = {}


# ----------------------------------------------------------------------------
# host-side gather (reference _rcds) and scatter (reference _merge), numpy
# ----------------------------------------------------------------------------

def _rcds_np(f0, f1, s, i):
    fr = np.concatenate([f0, f1], axis=3)
    fl = np.concatenate([f1, f0], axis=3)
    fb = np.concatenate([f0, f1], axis=2)
    ft = np.concatenate([f1, f0], axis=2)
    Bb, Cc = fr.shape[:2]
    r, l, b, t = OFF[i]
    y0 = fr[:, :, r[0]::s, r[1]::s].transpose(0, 1, 3, 2).reshape(Bb, Cc, -1)
    y1 = fl[:, :, l[0]::s, l[1]::s].transpose(0, 1, 3, 2).reshape(Bb, Cc, -1)[:, :, ::-1]
    y2 = fb[:, :, b[0]::s, b[1]::s].reshape(Bb, Cc, -1)
    y3 = ft[:, :, t[0]::s, t[1]::s].reshape(Bb, Cc, -1)[:, :, ::-1]
    feats = np.stack([y0, y1, y2, y3], axis=1)  # (B, 4, C, L)
    return np.ascontiguousarray(feats), fr.shape[2], fb.shape[3]


def _merge_np(ys, ori_h, ori_w, s, i):
    # ys: (B, 4, C, L)
    Bb, K, Cc, Ll = ys.shape
    Hh = -(-ori_h // s)
    Ww = -(-ori_w // s)
    nh, nw = Hh * s, Ww * s
    r, l, b, t = OFF[i]
    y2wr = np.zeros((Bb, Cc, nh, 2 * nw), ys.dtype)
    y2wl = np.zeros((Bb, Cc, nh, 2 * nw), ys.dtype)
    y2hb = np.zeros((Bb, Cc, 2 * nh, nw), ys.dtype)
    y2ht = np.zeros((Bb, Cc, 2 * nh, nw), ys.dtype)
    y2wr[:, :, r[0]::s, r[1]::s] = ys[:, 0].reshape(Bb, Cc, 2 * Ww, Hh).transpose(0, 1, 3, 2)
    y2wl[:, :, l[0]::s, l[1]::s] = ys[:, 1][:, :, ::-1].reshape(Bb, Cc, 2 * Ww, Hh).transpose(0, 1, 3, 2)
    y2hb[:, :, b[0]::s, b[1]::s] = ys[:, 2].reshape(Bb, Cc, 2 * Hh, Ww)
    y2ht[:, :, t[0]::s, t[1]::s] = ys[:, 3][:, :, ::-1].reshape(Bb, Cc, 2 * Hh, Ww)
    if ori_h != nh or ori_w != nw:
        y2wr = y2wr[:, :, :ori_h, :ori_w]
        y2wl = y2wl[:, :, :ori_h, :ori_w]
        y2ht = y2ht[:, :, :ori_h, :ori_w]
        y2hb = y2hb[:, :, :ori_h, :ori_w]
    d0r, d1r = np.split(y2wr, 2, axis=3)
    d1l, d0l = np.split(y2wl, 2, axis=3)
    d0b, d1b = np.split(y2hb, 2, axis=2)
    d1t, d0t = np.split(y2ht, 2, axis=2)
    return d0r + d0l + d0b + d0t, d1r + d1l + d1b + d1t


# ----------------------------------------------------------------------------
# device program
# ----------------------------------------------------------------------------

def _build_program():
    import concourse.bacc as bacc
    import concourse.mybir as mybir
    import concourse.tile as tile

    dt_ = mybir.dt
    F32, BF16, F32R = dt_.float32, dt_.bfloat16, dt_.float32r
    AF = mybir.ActivationFunctionType
    OP = mybir.AluOpType

    def r32(ap):
        return ap.bitcast(F32R)

    nc = bacc.Bacc("TRN2", target_bir_lowering=False, debug=False)

    def din(name, shape, d=F32):
        return nc.dram_tensor(name, shape, d, kind="ExternalInput")

    x_d = din("x", [C, L])
    W2T_d = din("W2T", [C, 2 * D_INNER])
    w1n_d = din("w1n", [1, 2 * D_INNER])
    diag_d = din("diag", [D_INNER, 4 * D_INNER])
    cbn_d = din("cbn", [D_INNER, 1])
    cbp_d = din("cbp", [D_INNER, 1])
    bzn_d = din("bzn", [D_INNER, 1])
    bzp_d = din("bzp", [D_INNER, 1])
    Wxp_d = din("Wxp", [D_INNER, DT_RANK + 2 * D_STATE])
    WdtT_d = din("WdtT", [DT_RANK, D_INNER])
    dtb_d = din("dtb", [D_INNER, 1])
    Aneg_d = din("Aneg", [D_INNER, D_STATE])
    Dsk_d = din("Dsk", [D_INNER, 1])
    WoutT_d = din("WoutT", [D_INNER, C])
    Wfc1T_d = din("Wfc1T", [C, 4 * C])
    w1f_d = din("w1f", [1, 4 * C])
    bfc1_d = din("bfc1", [4 * C // 2, 2])
    Wfc2a_d = din("Wfc2a", [2 * C, C])
    Wfc2b_d = din("Wfc2b", [2 * C, C])
    mv_d = din("mv", [C, 1])
    mvb_d = din("mvb", [C, 1], dt_.bfloat16)
    ones1_d = din("ones1", [1, C])
    epsb_d = din("epsb", [16, 1])
    I128_d = din("I128", [D_INNER, D_INNER], dt_.bfloat16)

    out_d = nc.dram_tensor("out", [C, L], F32, kind="ExternalOutput")

    NS = D_STATE

    with tile.TileContext(nc) as tc:
        with tc.tile_pool(name="pers", bufs=1) as pers, \
             tc.tile_pool(name="wka", bufs=2) as wka, \
             tc.tile_pool(name="wkb", bufs=3) as wkb, \
             tc.tile_pool(name="wk2", bufs=1) as wk2, \
             tc.tile_pool(name="hp", bufs=2) as hp, \
             tc.tile_pool(name="cvp", bufs=2) as cvp, \
             tc.tile_pool(name="bgp", bufs=2) as bgp, \
             tc.tile_pool(name="dap", bufs=2) as dap, \
             tc.tile_pool(name="dbp", bufs=2) as dbp, \
             tc.tile_pool(name="ps", bufs=5, space="PSUM") as ps, \
             tc.tile_pool(name="pso", bufs=1, space="PSUM") as pso, \
             tc.tile_pool(name="pacc", bufs=2, space="PSUM") as pacc, \
             tc.tile_pool(name="dr", bufs=1, space="DRAM") as dr:

            def ld(dram, shape, d=F32, tag=None, rr=False):
                t_ = pers.tile(shape, F32R if rr else d, tag=tag)
                if rr:
                    nc.sync.dma_start(t_[:], r32(dram[:]))
                else:
                    nc.sync.dma_start(t_[:], dram[:])
                return t_

            W2T = ld(W2T_d, [C, 2 * D_INNER], tag="W2T", rr=True)
            w1n = ld(w1n_d, [1, 2 * D_INNER], tag="w1n", rr=True)
            diag = ld(diag_d, [D_INNER, 4 * D_INNER], tag="diag", rr=True)
            cbn = ld(cbn_d, [D_INNER, 1], tag="cbn")
            cbp = ld(cbp_d, [D_INNER, 1], tag="cbp")
            bzn = ld(bzn_d, [D_INNER, 1], tag="bzn")
            bzp = ld(bzp_d, [D_INNER, 1], tag="bzp")
            Wxp = ld(Wxp_d, [D_INNER, DT_RANK + 2 * D_STATE], tag="Wxp", rr=True)
            WdtT = ld(WdtT_d, [DT_RANK, D_INNER], tag="WdtT", rr=True)
            dtb = ld(dtb_d, [D_INNER, 1], tag="dtb")
            Aneg = ld(Aneg_d, [D_INNER, D_STATE], tag="Aneg")
            Dsk = ld(Dsk_d, [D_INNER, 1], tag="Dsk")
            WoutT = ld(WoutT_d, [D_INNER, C], tag="WoutT", rr=True)
            Wfc1T = ld(Wfc1T_d, [C, 4 * C], tag="Wfc1T", rr=True)
            w1f = ld(w1f_d, [1, 4 * C], tag="w1f", rr=True)
            bfc1 = ld(bfc1_d, [4 * C // 2, 2], tag="bfc1")
            Wfc2a = ld(Wfc2a_d, [2 * C, C], tag="Wfc2a", rr=True)
            Wfc2b = ld(Wfc2b_d, [2 * C, C], tag="Wfc2b", rr=True)
            mv = ld(mv_d, [C, 1], tag="mv", rr=True)
            mvb = ld(mvb_d, [C, 1], BF16, tag="mvb")
            ones1 = ld(ones1_d, [1, C], tag="ones1", rr=True)
            epsb = ld(epsb_d, [16, 1], tag="epsb")
            I128 = ld(I128_d, [D_INNER, D_INNER], BF16, tag="I128")

            fbuf = pers.tile([C, L], BF16, tag="fbuf")

            st1_dram = dr.tile([32, T], F32, tag="st1d")
            r1_dram = dr.tile([1, L], F32, tag="r1d")
            q1_dram = dr.tile([1, L], F32, tag="q1d")
            st2_dram = dr.tile([32, T], F32, tag="st2d")
            r2_dram = dr.tile([1, L], F32, tag="r2d")
            q2_dram = dr.tile([1, L], F32, tag="q2d")
            # chunk-major so broadcast-source rows are contiguous in DRAM
            bc_dram = dr.tile([NCH * 2 * NS, T], BF16, tag="bcd")

            # ================= phase 0: LN1 stats =================
            for j in range(NCH):
                sl = slice(j * T, (j + 1) * T)
                xj = wka.tile([C, T], F32, tag="xj")
                nc.sync.dma_start(r32(xj[:]), r32(x_d[:, sl]))
                x2 = wka.tile([C, T], F32, tag="f2")
                nc.scalar.activation(r32(x2[:]), xj[:], AF.Square)
                mu_ps = ps.tile([1, T], F32, tag="ps")
                nc.tensor.matmul(mu_ps[:], mv[:], r32(xj[:]), start=True, stop=True)
                ms_ps = ps.tile([1, T], F32, tag="ps")
                nc.tensor.matmul(ms_ps[:], mv[:], r32(x2[:]), start=True, stop=True)
                mu_sb = wka.tile([1, T], F32, tag="msb")
                nc.scalar.copy(mu_sb[:], mu_ps[:])
                ms_sb = wka.tile([1, T], F32, tag="ssb")
                nc.scalar.copy(ms_sb[:], ms_ps[:])
                nc.sync.dma_start(st1_dram[j:j + 1, :], mu_sb[:])
                nc.sync.dma_start(st1_dram[16 + j:17 + j, :], ms_sb[:])

            def stats_finish(st_dram, rf_dram, qf_dram):
                mu_all = wk2.tile([NCH, T], F32, tag="sa")
                nc.sync.dma_start(mu_all[:], st_dram[0:16, :])
                ms_all = wk2.tile([NCH, T], F32, tag="sb")
                nc.sync.dma_start(ms_all[:], st_dram[16:32, :])
                t1 = wk2.tile([NCH, T], F32, tag="sc")
                nc.scalar.activation(t1[:], mu_all[:], AF.Square)
                t2 = wk2.tile([NCH, T], F32, tag="sd")
                nc.vector.tensor_sub(t2[:], ms_all[:], t1[:])
                t3 = wk2.tile([NCH, T], F32, tag="sc")
                nc.scalar.activation(t3[:], t2[:], AF.Ln, bias=epsb[:])
                rstd = wk2.tile([NCH, T], F32, tag="sd")
                nc.scalar.activation(rstd[:], t3[:], AF.Exp, bias=0.0, scale=-0.5)
                mq = wk2.tile([NCH, T], F32, tag="sc")
                nc.vector.tensor_mul(mq[:], mu_all[:], rstd[:])
                nc.sync.dma_start(rf_dram[:].rearrange("a (c t) -> (a c) t", t=T), rstd[:])
                nc.sync.dma_start(qf_dram[:].rearrange("a (c t) -> (a c) t", t=T), mq[:])

            stats_finish(st1_dram, r1_dram, q1_dram)

            # ================= steady phase =================
            G = 4   # states per tile group
            GD = 2  # states per broadcast DMA (descriptor-size tradeoff)
            NG = NS // G
            prev_h = None
            prev_cv = None
            for j in range(NCH):
                sl = slice(j * T, (j + 1) * T)
                bcr = j * 2 * NS  # chunk-major row base in bc_dram
                xj = wka.tile([C, T], F32, tag="xj")
                nc.sync.dma_start(xj[:], x_d[:, sl])
                rft = wka.tile([1, T], F32, tag="rft")
                nc.sync.dma_start(r32(rft[:]), r32(r1_dram[0:1, sl]))
                qft = wka.tile([1, T], F32, tag="qft")
                nc.sync.dma_start(r32(qft[:]), r32(q1_dram[0:1, sl]))

                rb = ps.tile([C, T], F32, tag="ps")
                nc.tensor.matmul(rb[:], ones1[:], r32(rft[:]), start=True, stop=True)
                xr = wka.tile([C, T], F32, tag="xr")
                nc.vector.tensor_mul(r32(xr[:]), xj[:], rb[:])

                xzh = ps.tile([D_INNER, T], F32, tag="ps")
                nc.tensor.matmul(xzh[:], r32(W2T[:, 0:D_INNER]), r32(xr[:]),
                                 start=True, stop=False, skip_group_check=True)
                nc.tensor.matmul(xzh[:], r32(w1n[:, 0:D_INNER]), r32(qft[:]),
                                 start=False, stop=True, skip_group_check=True)
                xzz = ps.tile([D_INNER, T], F32, tag="ps")
                nc.tensor.matmul(xzz[:], r32(W2T[:, D_INNER:2 * D_INNER]), r32(xr[:]),
                                 start=True, stop=False, skip_group_check=True)
                nc.tensor.matmul(xzz[:], r32(w1n[:, D_INNER:2 * D_INNER]), r32(qft[:]),
                                 start=False, stop=True, skip_group_check=True)

                cv = cvp.tile([D_INNER, T + 3], F32, tag="cv")
                if j == 0:
                    nc.vector.memset(cv[:, 0:3], 0.0)
                else:
                    nc.vector.tensor_copy(r32(cv[:, 0:3]), r32(prev_cv[:, T:T + 3]))
                nc.scalar.activation(r32(cv[:, 3:T + 3]), xzh[:], AF.Copy)
                cps = ps.tile([D_INNER, T], F32, tag="ps")
                for k in range(4):
                    nc.tensor.matmul(cps[:], r32(diag[:, k * D_INNER:(k + 1) * D_INNER]),
                                     r32(cv[:, k:k + T]),
                                     start=(k == 0), stop=(k == 3), skip_group_check=True)
                prev_cv = cv

                # both silus via sigmoid = exp(-ln(1+e^-u)); Exp/Ln ops clustered
                # to minimize activation-table switches
                ez = wka.tile([D_INNER, T], F32, tag="tmpa")
                nc.scalar.activation(ez[:], xzz[:], AF.Exp, bias=bzn[:], scale=-1.0)
                ec = wka.tile([D_INNER, T], F32, tag="tmpc")
                nc.scalar.activation(ec[:], cps[:], AF.Exp, bias=cbn[:], scale=-1.0)
                nc.vector.tensor_scalar_add(ez[:], ez[:], 1.0)
                nc.vector.tensor_scalar_add(ec[:], ec[:], 1.0)
                lz = wka.tile([D_INNER, T], F32, tag="tmpb")
                nc.scalar.activation(lz[:], ez[:], AF.Ln)
                lc = wka.tile([D_INNER, T], F32, tag="tmpd")
                nc.scalar.activation(lc[:], ec[:], AF.Ln)
                gc = wka.tile([D_INNER, T], F32, tag="tmpc")
                nc.scalar.activation(gc[:], lc[:], AF.Exp, bias=0.0, scale=-1.0)
                gz = wka.tile([D_INNER, T], F32, tag="tmpa")
                nc.scalar.activation(gz[:], lz[:], AF.Exp, bias=0.0, scale=-1.0)
                xh = wka.tile([D_INNER, T], F32, tag="xh")
                nc.vector.scalar_tensor_tensor(r32(xh[:]), cps[:], cbp[:], gc[:],
                                               OP.add, OP.mult)
                sz = wka.tile([D_INNER, T], F32, tag="sz")
                nc.vector.scalar_tensor_tensor(sz[:], xzz[:], bzp[:], gz[:], OP.add, OP.mult)

                dblA = ps.tile([DT_RANK, T], F32, tag="ps")
                nc.tensor.matmul(dblA[:], r32(Wxp[:, 0:DT_RANK]), r32(xh[:]),
                                 start=True, stop=True)
                dblB = ps.tile([2 * NS, T], F32, tag="ps")
                nc.tensor.matmul(dblB[:], r32(Wxp[:, DT_RANK:]), r32(xh[:]),
                                 start=True, stop=True)
                dtin = wkb.tile([DT_RANK, T], F32, tag="dtin")
                nc.scalar.activation(r32(dtin[:]), dblA[:], AF.Copy)
                bc = wkb.tile([2 * NS, T], BF16, tag="bc")
                nc.scalar.copy(bc[:], dblB[:])
                nc.sync.dma_start(bc_dram[bcr:bcr + 2 * NS, :], bc[:])

                # broadcast B/C state-rows to all partitions; contiguous 2-row
                # groups in chunk-major bc_dram -> 1 descriptor per partition
                Bgs, Cgs = [], []
                for g in range(NG):
                    Bg = bgp.tile([D_INNER, G, T], BF16, tag="Bg")
                    for h in range(G // GD):
                        r0 = bcr + g * G + h * GD
                        nc.sync.dma_start(
                            Bg[:, h * GD:(h + 1) * GD, :],
                            bc_dram[r0:r0 + GD, :][None]
                            .broadcast_to([D_INNER, GD, T]))
                    Bgs.append(Bg)
                    Cg = bgp.tile([D_INNER, G, T], BF16, tag="Cg")
                    for h in range(G // GD):
                        r0 = bcr + NS + g * G + h * GD
                        nc.sync.dma_start(
                            Cg[:, h * GD:(h + 1) * GD, :],
                            bc_dram[r0:r0 + GD, :][None]
                            .broadcast_to([D_INNER, GD, T]))
                    Cgs.append(Cg)

                dtp = ps.tile([D_INNER, T], F32, tag="ps")
                nc.tensor.matmul(dtp[:], r32(WdtT[:]), r32(dtin[:]), start=True, stop=True)
                esp = wka.tile([D_INNER, T], F32, tag="tmpa")
                nc.scalar.activation(esp[:], dtp[:], AF.Exp, bias=dtb[:], scale=1.0)
                nc.vector.tensor_scalar_add(esp[:], esp[:], 1.0)
                dt = wka.tile([D_INNER, T], F32, tag="dt")
                nc.scalar.activation(dt[:], esp[:], AF.Ln)

                w = wkb.tile([D_INNER, T], BF16, tag="w")
                nc.vector.tensor_mul(w[:], dt[:], xh[:])

                cur_h = hp.tile([D_INNER, NS, T], BF16, tag="h")
                yacc = pacc.tile([D_INNER, T], F32, tag="yacc")
                for g in range(NG):
                    dAg = dap.tile([D_INNER, G, T], F32, tag="dAg")
                    for k in range(G):
                        n = g * G + k
                        nc.scalar.activation(dAg[:, k, :], dt[:], AF.Exp, bias=0.0,
                                             scale=Aneg[:, n:n + 1])
                    dBg = dbp.tile([D_INNER, G, T], BF16, tag="dBg")
                    nc.vector.tensor_tensor(dBg[:], w[:][:, None].broadcast_to(
                        [D_INNER, G, T]), Bgs[g][:], OP.mult)
                    for k in range(G):
                        n = g * G + k
                        init = 0.0 if j == 0 else prev_h[:, n, T - 1:T]
                        nc.vector.tensor_tensor_scan(cur_h[:, n, :], dAg[:, k, :],
                                                     dBg[:, k, :], init,
                                                     OP.mult, OP.add)
                    Pg = dbp.tile([D_INNER, G, T], BF16, tag="Pg")
                    nc.vector.tensor_tensor(Pg[:], cur_h[:, g * G:(g + 1) * G, :],
                                            Cgs[g][:], OP.mult)
                    for k in range(G):
                        n = g * G + k
                        nc.tensor.matmul(yacc[:], I128[:], Pg[:, k, :], start=(n == 0),
                                         stop=(n == NS - 1), skip_group_check=True)
                prev_h = cur_h

                y = wka.tile([D_INNER, T], F32, tag="y")
                nc.vector.scalar_tensor_tensor(y[:], xh[:], Dsk[:], yacc[:],
                                               OP.mult, OP.add)
                gated = wka.tile([D_INNER, T], F32, tag="gated")
                nc.vector.tensor_mul(r32(gated[:]), y[:], sz[:])
                opj = pso.tile([C, T], F32, tag="pso")
                nc.tensor.matmul(opj[:], r32(WoutT[:]), r32(gated[:]), start=True, stop=True)
                nc.vector.tensor_add(fbuf[:, sl], xj[:], opj[:])

                f2 = wka.tile([C, T], F32, tag="f2")
                nc.scalar.activation(r32(f2[:]), fbuf[:, sl], AF.Square)
                mu2_ps = pso.tile([1, T], F32, tag="pso")
                nc.tensor.matmul(mu2_ps[:], mvb[:], fbuf[:, sl], start=True, stop=True)
                ms2_ps = pso.tile([1, T], F32, tag="pso")
                nc.tensor.matmul(ms2_ps[:], mv[:], r32(f2[:]), start=True, stop=True)
                mu2_sb = wka.tile([1, T], F32, tag="msb")
                nc.scalar.copy(mu2_sb[:], mu2_ps[:])
                ms2_sb = wka.tile([1, T], F32, tag="ssb")
                nc.scalar.copy(ms2_sb[:], ms2_ps[:])
                nc.sync.dma_start(st2_dram[j:j + 1, :], mu2_sb[:])
                nc.sync.dma_start(st2_dram[16 + j:17 + j, :], ms2_sb[:])

            stats_finish(st2_dram, r2_dram, q2_dram)

            # ================= final phase: LN2 + MLP (gelu table) =========
            for j in range(NCH):
                sl = slice(j * T, (j + 1) * T)
                rft2 = wka.tile([1, T], F32, tag="rft")
                nc.sync.dma_start(r32(rft2[:]), r32(r2_dram[0:1, sl]))
                qft2 = wka.tile([1, T], F32, tag="qft")
                nc.sync.dma_start(r32(qft2[:]), r32(q2_dram[0:1, sl]))
                rb2 = ps.tile([C, T], F32, tag="ps")
                nc.tensor.matmul(rb2[:], ones1[:], r32(rft2[:]), start=True, stop=True)
                fr = wka.tile([C, T], F32, tag="fr")
                nc.vector.tensor_mul(r32(fr[:]), fbuf[:, sl], rb2[:])
                gtiles = []
                for h in range(2):
                    gp = ps.tile([2 * C, T], F32, tag="ps")
                    nc.tensor.matmul(gp[:], r32(Wfc1T[:, h * 2 * C:(h + 1) * 2 * C]),
                                     r32(fr[:]), start=True, stop=False,
                                     skip_group_check=True)
                    nc.tensor.matmul(gp[:], r32(w1f[:, h * 2 * C:(h + 1) * 2 * C]),
                                     r32(qft2[:]), start=False, stop=True,
                                     skip_group_check=True)
                    g = wka.tile([2 * C, T], F32, tag="g")
                    nc.scalar.activation(r32(g[:]), gp[:], AF.Gelu, bias=bfc1[:, h:h + 1])
                    gtiles.append(g)
                f2p = ps.tile([C, T], F32, tag="ps")
                nc.tensor.matmul(f2p[:], r32(Wfc2a[:]), r32(gtiles[0][:]),
                                 start=True, stop=False, skip_group_check=True)
                nc.tensor.matmul(f2p[:], r32(Wfc2b[:]), r32(gtiles[1][:]),
                                 start=False, stop=True, skip_group_check=True)
                outf = wka.tile([C, T], F32, tag="outf")
                nc.vector.tensor_add(outf[:], fbuf[:, sl], f2p[:])
                nc.sync.dma_start(out_d[:, sl], outf[:])

    nc.compile()
    return nc


def _get_program():
    if "nc" not in _PROG:
        _PROG["nc"] = _build_program()
    return _PROG["nc"]


# ----------------------------------------------------------------------------
# host weight preprocessing per direction
# ----------------------------------------------------------------------------

def _bf16_dtype():
    try:
        import ml_dtypes
        return ml_dtypes.bfloat16
    except ImportError:
        import jax.numpy as jnp
        return jnp.bfloat16


def _prep_weights(li, inputs):
    f32 = np.float32
    in_w = np.asarray(inputs["in_proj_w"][li], np.float64)
    nw = np.asarray(inputs["norm_w"][li], np.float64)
    nb = np.asarray(inputs["norm_b"][li], np.float64)
    W2 = in_w * nw[None, :]
    bz_full = in_w @ nb
    b_h, b_z = bz_full[:D_INNER], bz_full[D_INNER:]
    cw = np.asarray(inputs["conv_w"][li], np.float64)
    cb = np.asarray(inputs["conv_b"][li], np.float64)
    cbtot = cb + b_h * cw.sum(1)
    diag = np.zeros((D_INNER, 4, D_INNER), np.float64)
    kk = np.arange(D_INNER)
    for k in range(4):
        diag[kk, k, kk] = cw[:, k]
    xp = np.asarray(inputs["x_proj_w"][li], np.float64)
    fc1 = np.asarray(inputs["fc1_w"], np.float64)
    fw = np.asarray(inputs["fnorm_w"], np.float64)
    fb = np.asarray(inputs["fnorm_b"], np.float64)
    fc1p = fc1 * fw[None, :]
    bfc1 = fc1 @ fb
    fc2 = np.asarray(inputs["fc2_w"], np.float64)
    return {
        "W2T": W2.T.astype(f32),
        "w1n": (-W2.sum(1))[None, :].astype(f32),
        "diag": diag.reshape(D_INNER, 4 * D_INNER).astype(f32),
        "cbn": (-cbtot)[:, None].astype(f32),
        "cbp": cbtot[:, None].astype(f32),
        "bzn": (-b_z)[:, None].astype(f32),
        "bzp": b_z[:, None].astype(f32),
        "Wxp": xp.T.astype(f32),
        "WdtT": np.asarray(inputs["dt_proj_w"][li], np.float64).T.astype(f32),
        "dtb": np.asarray(inputs["dt_proj_b"][li], f32)[:, None],
        "Aneg": (-np.exp(np.asarray(inputs["A_log"][li], np.float64))).astype(f32),
        "Dsk": np.asarray(inputs["D_skip"][li], f32)[:, None],
        "WoutT": np.asarray(inputs["out_proj_w"][li], np.float64).T.astype(f32),
        "Wfc1T": fc1p.T.astype(f32),
        "w1f": (-fc1p.sum(1))[None, :].astype(f32),
        "bfc1": bfc1.reshape(2, 128).T.astype(f32),
        "Wfc2a": fc2.T[:2 * C].astype(f32),
        "Wfc2b": fc2.T[2 * C:].astype(f32),
        "mv": np.full((C, 1), 1.0 / C, f32),
        "mvb": np.full((C, 1), 1.0 / C).astype(_bf16_dtype()),
        "ones1": np.ones((1, C), f32),
        "epsb": np.full((16, 1), EPS, f32),
        "I128": np.eye(D_INNER).astype(_bf16_dtype()),
    }


def _reference_np(**inputs):
    """Pure-numpy fallback replica of the reference (slow, exact)."""
    i = int(inputs["src_number"]) % 4
    s = int(inputs["step_size"])
    feats, ori_h, ori_w = _rcds_np(np.asarray(inputs["ref_feat"], np.float32),
                                   np.asarray(inputs["src_feat"], np.float32), s, i)
    Bb, K, Cc, Ll = feats.shape
    f = feats.astype(np.float64)
    outs = np.empty_like(f)
    for d in range(4):
        li = d
        x = f[:, d].transpose(0, 2, 1)  # (B,L,C)
        mu = x.mean(-1, keepdims=True)
        var = ((x - mu) ** 2).mean(-1, keepdims=True)
        h = (x - mu) / np.sqrt(var + EPS) * np.asarray(inputs["norm_w"][li]) \
            + np.asarray(inputs["norm_b"][li])
        xz = h @ np.asarray(inputs["in_proj_w"][li]).T
        xh, z = xz[..., :D_INNER], xz[..., D_INNER:]
        xpd = np.pad(xh.transpose(0, 2, 1), ((0, 0), (0, 0), (3, 0)))
        cw = np.asarray(inputs["conv_w"][li])
        xc = sum(cw[:, k:k + 1] * xpd[:, :, k:k + Ll] for k in range(4))
        xc = xc + np.asarray(inputs["conv_b"][li])[None, :, None]
        xh = (xc / (1 + np.exp(-xc))).transpose(0, 2, 1)
        dbl = xh @ np.asarray(inputs["x_proj_w"][li]).T
        dtv = dbl[..., :DT_RANK]
        Bm = dbl[..., DT_RANK:DT_RANK + D_STATE]
        Cm = dbl[..., DT_RANK + D_STATE:]
        dtp = dtv @ np.asarray(inputs["dt_proj_w"][li]).T + np.asarray(inputs["dt_proj_b"][li])
        dtv = np.logaddexp(0, dtp)
        A = -np.exp(np.asarray(inputs["A_log"][li], np.float64))
        dA = np.exp(dtv[..., None] * A)
        dBu = (dtv * xh)[..., None] * Bm[:, :, None, :]
        hst = np.zeros((Bb, D_INNER, D_STATE))
        ys = np.empty((Bb, Ll, D_INNER))
        for t in range(Ll):
            hst = dA[:, t] * hst + dBu[:, t]
            ys[:, t] = np.einsum("bdn,bn->bd", hst, Cm[:, t])
        ys = ys + xh * np.asarray(inputs["D_skip"][li])
        ys = ys * (z / (1 + np.exp(-z)))
        o = ys @ np.asarray(inputs["out_proj_w"][li]).T
        outs[:, d] = (x + o).transpose(0, 2, 1)
    x = outs.transpose(0, 1, 3, 2)  # (B,4,L,C)
    mu = x.mean(-1, keepdims=True)
    var = ((x - mu) ** 2).mean(-1, keepdims=True)
    h = (x - mu) / np.sqrt(var + EPS) * np.asarray(inputs["fnorm_w"]) \
        + np.asarray(inputs["fnorm_b"])
    from scipy.special import erf
    g = h @ np.asarray(inputs["fc1_w"]).T
    g = 0.5 * g * (1 + erf(g / np.sqrt(2)))
    x = x + g @ np.asarray(inputs["fc2_w"]).T
    d0, d1 = _merge_np(x.transpose(0, 1, 3, 2).astype(np.float32), ori_h, ori_w, s, i)
    return d0.astype(np.float32), d1.astype(np.float32)


def kernel(**inputs):
    s = int(inputs["step_size"])
    i = int(inputs["src_number"]) % 4
    ref_feat = np.asarray(inputs["ref_feat"], np.float32)
    src_feat = np.asarray(inputs["src_feat"], np.float32)

    if s != 2 or ref_feat.shape != (B, C, HIMG, WIMG):
        return _reference_np(**inputs)

    feats, ori_h, ori_w = _rcds_np(ref_feat, src_feat, s, i)  # (B,4,C,L)

    wmaps = [_prep_weights(d, inputs) for d in range(4)]
    in_maps = []
    for core in range(8):
        b, d = core // 4, core % 4
        m = dict(wmaps[d])
        m["x"] = np.ascontiguousarray(feats[b, d])
        in_maps.append(m)

    from concourse.bass_utils import run_bass_kernel_spmd
    nc = _get_program()
    res = run_bass_kernel_spmd(nc, in_maps, list(range(8)))

    ys = np.empty((B, 4, C, L), np.float32)
    for core in range(8):
        b, d = core // 4, core % 4
        ys[b, d] = res.results[core]["out"]

    d0, d1 = _merge_np(ys, ori_h, ori_w, s, i)
    return d0, d1



# revision 16
# speedup vs baseline: 1.7106x; 1.0132x over previous
"""Trainium2 Bass kernel for nn_DM_35210141892754 (4-direction VMamba block).

Sharding: 8 cores = B(2) x directions(4); each core processes one gathered
sequence (C=64, L=8192) end-to-end. Gather/scatter (reference _rcds/_merge)
run on host with numpy strided ops; all FLOPs run on device.
"""

import sys
import math

sys.path.insert(0, "/opt/trn_rl_repo")

import numpy as np

B, C, HIMG, WIMG = 2, 64, 128, 128
DEPTH = 4
D_STATE, D_CONV, EXPAND = 16, 4, 2
D_INNER = EXPAND * C  # 128
DT_RANK = math.ceil(C / 16)  # 4
L = 8192  # sequence length for step_size=2
T = 512  # device chunk size
NCH = L // T
EPS = 1e-5

OFF = {0: ((1, 0), (0, 0), (0, 1), (1, 1)),
       1: ((0, 0), (1, 0), (1, 1), (0, 1)),
       2: ((0, 1), (1, 1), (1, 0), (0, 0)),
       3: ((1, 1), (0, 1), (0, 0), (1, 0))}

_PROG = {}


# ----------------------------------------------------------------------------
# host-side gather (reference _rcds) and scatter (reference _merge), numpy
# ----------------------------------------------------------------------------

def _rcds_np(f0, f1, s, i):
    fr = np.concatenate([f0, f1], axis=3)
    fl = np.concatenate([f1, f0], axis=3)
    fb = np.concatenate([f0, f1], axis=2)
    ft = np.concatenate([f1, f0], axis=2)
    Bb, Cc = fr.shape[:2]
    r, l, b, t = OFF[i]
    y0 = fr[:, :, r[0]::s, r[1]::s].transpose(0, 1, 3, 2).reshape(Bb, Cc, -1)
    y1 = fl[:, :, l[0]::s, l[1]::s].transpose(0, 1, 3, 2).reshape(Bb, Cc, -1)[:, :, ::-1]
    y2 = fb[:, :, b[0]::s, b[1]::s].reshape(Bb, Cc, -1)
    y3 = ft[:, :, t[0]::s, t[1]::s].reshape(Bb, Cc, -1)[:, :, ::-1]
    feats = np.stack([y0, y1, y2, y3], axis=1)  # (B, 4, C, L)
    return np.ascontiguousarray(feats), fr.shape[2], fb.shape[3]


def _merge_np(ys, ori_h, ori_w, s, i):
    # ys: (B, 4, C, L)
    Bb, K, Cc, Ll = ys.shape
    Hh = -(-ori_h // s)
    Ww = -(-ori_w // s)
    nh, nw = Hh * s, Ww * s
    r, l, b, t = OFF[i]
    y2wr = np.zeros((Bb, Cc, nh, 2 * nw), ys.dtype)
    y2wl = np.zeros((Bb, Cc, nh, 2 * nw), ys.dtype)
    y2hb = np.zeros((Bb, Cc, 2 * nh, nw), ys.dtype)
    y2ht = np.zeros((Bb, Cc, 2 * nh, nw), ys.dtype)
    y2wr[:, :, r[0]::s, r[1]::s] = ys[:, 0].reshape(Bb, Cc, 2 * Ww, Hh).transpose(0, 1, 3, 2)
    y2wl[:, :, l[0]::s, l[1]::s] = ys[:, 1][:, :, ::-1].reshape(Bb, Cc, 2 * Ww, Hh).transpose(0, 1, 3, 2)
    y2hb[:, :, b[0]::s, b[1]::s] = ys[:, 2].reshape(Bb, Cc, 2 * Hh, Ww)
    y2ht[:, :, t[0]::s, t[1]::s] = ys[:, 3][:, :, ::-1].reshape(Bb, Cc, 2 * Hh, Ww)
    if ori_h != nh or ori_w != nw:
        y2wr = y2wr[:, :, :ori_h, :ori_w]
        y2wl = y2wl[:, :, :ori_h, :ori_w]
        y2ht = y2ht[:, :, :ori_h, :ori_w]
        y2hb = y2hb[:, :, :ori_h, :ori_w]
    d0r, d1r = np.split(y2wr, 2, axis=3)
    d1l, d0l = np.split(y2wl, 2, axis=3)
    d0b, d1b = np.split(y2hb, 2, axis=2)
    d1t, d0t = np.split(y2ht, 2, axis=2)
    return d0r + d0l + d0b + d0t, d1r + d1l + d1b + d1t


# ----------------------------------------------------------------------------
# device program
# ----------------------------------------------------------------------------

def _build_program():
    import concourse.bacc as bacc
    import concourse.mybir as mybir
    import concourse.tile as tile

    dt_ = mybir.dt
    F32, BF16, F32R = dt_.float32, dt_.bfloat16, dt_.float32r
    AF = mybir.ActivationFunctionType
    OP = mybir.AluOpType

    def r32(ap):
        return ap.bitcast(F32R)

    nc = bacc.Bacc("TRN2", target_bir_lowering=False, debug=False)

    def din(name, shape, d=F32):
        return nc.dram_tensor(name, shape, d, kind="ExternalInput")

    x_d = din("x", [C, L])
    W2T_d = din("W2T", [C, 2 * D_INNER])
    w1n_d = din("w1n", [1, 2 * D_INNER])
    diag_d = din("diag", [D_INNER, 4 * D_INNER])
    cbn_d = din("cbn", [D_INNER, 1])
    cbp_d = din("cbp", [D_INNER, 1])
    bzn_d = din("bzn", [D_INNER, 1])
    bzp_d = din("bzp", [D_INNER, 1])
    Wxp_d = din("Wxp", [D_INNER, DT_RANK + 2 * D_STATE])
    WdtT_d = din("WdtT", [DT_RANK, D_INNER])
    dtb_d = din("dtb", [D_INNER, 1])
    Aneg_d = din("Aneg", [D_INNER, D_STATE])
    Dsk_d = din("Dsk", [D_INNER, 1])
    WoutT_d = din("WoutT", [D_INNER, C])
    Wfc1T_d = din("Wfc1T", [C, 4 * C])
    w1f_d = din("w1f", [1, 4 * C])
    bfc1_d = din("bfc1", [4 * C // 2, 2])
    Wfc2a_d = din("Wfc2a", [2 * C, C])
    Wfc2b_d = din("Wfc2b", [2 * C, C])
    mv_d = din("mv", [C, 1])
    mvb_d = din("mvb", [C, 1], dt_.bfloat16)
    ones1_d = din("ones1", [1, C])
    epsb_d = din("epsb", [16, 1])
    I128_d = din("I128", [D_INNER, D_INNER], dt_.bfloat16)

    out_d = nc.dram_tensor("out", [C, L], F32, kind="ExternalOutput")

    NS = D_STATE

    with tile.TileContext(nc) as tc:
        with tc.tile_pool(name="pers", bufs=1) as pers, \
             tc.tile_pool(name="wka", bufs=2) as wka, \
             tc.tile_pool(name="wkb", bufs=3) as wkb, \
             tc.tile_pool(name="wk2", bufs=1) as wk2, \
             tc.tile_pool(name="hp", bufs=2) as hp, \
             tc.tile_pool(name="cvp", bufs=2) as cvp, \
             tc.tile_pool(name="bgp", bufs=2) as bgp, \
             tc.tile_pool(name="dap", bufs=2) as dap, \
             tc.tile_pool(name="dbp", bufs=2) as dbp, \
             tc.tile_pool(name="ps", bufs=5, space="PSUM") as ps, \
             tc.tile_pool(name="pso", bufs=1, space="PSUM") as pso, \
             tc.tile_pool(name="pacc", bufs=2, space="PSUM") as pacc, \
             tc.tile_pool(name="dr", bufs=1, space="DRAM") as dr:

            def ld(dram, shape, d=F32, tag=None, rr=False):
                t_ = pers.tile(shape, F32R if rr else d, tag=tag)
                if rr:
                    nc.sync.dma_start(t_[:], r32(dram[:]))
                else:
                    nc.sync.dma_start(t_[:], dram[:])
                return t_

            W2T = ld(W2T_d, [C, 2 * D_INNER], tag="W2T", rr=True)
            w1n = ld(w1n_d, [1, 2 * D_INNER], tag="w1n", rr=True)
            diag = ld(diag_d, [D_INNER, 4 * D_INNER], tag="diag", rr=True)
            cbn = ld(cbn_d, [D_INNER, 1], tag="cbn")
            cbp = ld(cbp_d, [D_INNER, 1], tag="cbp")
            bzn = ld(bzn_d, [D_INNER, 1], tag="bzn")
            bzp = ld(bzp_d, [D_INNER, 1], tag="bzp")
            Wxp = ld(Wxp_d, [D_INNER, DT_RANK + 2 * D_STATE], tag="Wxp", rr=True)
            WdtT = ld(WdtT_d, [DT_RANK, D_INNER], tag="WdtT", rr=True)
            dtb = ld(dtb_d, [D_INNER, 1], tag="dtb")
            Aneg = ld(Aneg_d, [D_INNER, D_STATE], tag="Aneg")
            Dsk = ld(Dsk_d, [D_INNER, 1], tag="Dsk")
            WoutT = ld(WoutT_d, [D_INNER, C], tag="WoutT", rr=True)
            Wfc1T = ld(Wfc1T_d, [C, 4 * C], tag="Wfc1T", rr=True)
            w1f = ld(w1f_d, [1, 4 * C], tag="w1f", rr=True)
            bfc1 = ld(bfc1_d, [4 * C // 2, 2], tag="bfc1")
            Wfc2a = ld(Wfc2a_d, [2 * C, C], tag="Wfc2a", rr=True)
            Wfc2b = ld(Wfc2b_d, [2 * C, C], tag="Wfc2b", rr=True)
            mv = ld(mv_d, [C, 1], tag="mv", rr=True)
            mvb = ld(mvb_d, [C, 1], BF16, tag="mvb")
            ones1 = ld(ones1_d, [1, C], tag="ones1", rr=True)
            epsb = ld(epsb_d, [16, 1], tag="epsb")
            I128 = ld(I128_d, [D_INNER, D_INNER], BF16, tag="I128")

            fbuf = pers.tile([C, L], BF16, tag="fbuf")

            st1_dram = dr.tile([32, T], F32, tag="st1d")
            r1_dram = dr.tile([1, L], F32, tag="r1d")
            q1_dram = dr.tile([1, L], F32, tag="q1d")
            st2_dram = dr.tile([32, T], F32, tag="st2d")
            r2_dram = dr.tile([1, L], F32, tag="r2d")
            q2_dram = dr.tile([1, L], F32, tag="q2d")
            # chunk-major so broadcast-source rows are contiguous in DRAM
            bc_dram = dr.tile([NCH * 2 * NS, T], BF16, tag="bcd")

            # ================= phase 0: LN1 stats =================
            for j in range(NCH):
                sl = slice(j * T, (j + 1) * T)
                xj = wka.tile([C, T], F32, tag="xj")
                nc.sync.dma_start(r32(xj[:]), r32(x_d[:, sl]))
                x2 = wka.tile([C, T], F32, tag="f2")
                nc.scalar.activation(r32(x2[:]), xj[:], AF.Square)
                mu_ps = ps.tile([1, T], F32, tag="ps")
                nc.tensor.matmul(mu_ps[:], mv[:], r32(xj[:]), start=True, stop=True)
                ms_ps = ps.tile([1, T], F32, tag="ps")
                nc.tensor.matmul(ms_ps[:], mv[:], r32(x2[:]), start=True, stop=True)
                mu_sb = wka.tile([1, T], F32, tag="msb")
                nc.vector.tensor_copy(mu_sb[:], mu_ps[:])
                ms_sb = wka.tile([1, T], F32, tag="ssb")
                nc.vector.tensor_copy(ms_sb[:], ms_ps[:])
                nc.sync.dma_start(st1_dram[j:j + 1, :], mu_sb[:])
                nc.sync.dma_start(st1_dram[16 + j:17 + j, :], ms_sb[:])

            def stats_finish(st_dram, rf_dram, qf_dram):
                mu_all = wk2.tile([NCH, T], F32, tag="sa")
                nc.sync.dma_start(mu_all[:], st_dram[0:16, :])
                ms_all = wk2.tile([NCH, T], F32, tag="sb")
                nc.sync.dma_start(ms_all[:], st_dram[16:32, :])
                t1 = wk2.tile([NCH, T], F32, tag="sc")
                nc.scalar.activation(t1[:], mu_all[:], AF.Square)
                t2 = wk2.tile([NCH, T], F32, tag="sd")
                nc.vector.tensor_sub(t2[:], ms_all[:], t1[:])
                t3 = wk2.tile([NCH, T], F32, tag="sc")
                nc.scalar.activation(t3[:], t2[:], AF.Ln, bias=epsb[:])
                rstd = wk2.tile([NCH, T], F32, tag="sd")
                nc.scalar.activation(rstd[:], t3[:], AF.Exp, bias=0.0, scale=-0.5)
                mq = wk2.tile([NCH, T], F32, tag="sc")
                nc.vector.tensor_mul(mq[:], mu_all[:], rstd[:])
                nc.sync.dma_start(rf_dram[:].rearrange("a (c t) -> (a c) t", t=T), rstd[:])
                nc.sync.dma_start(qf_dram[:].rearrange("a (c t) -> (a c) t", t=T), mq[:])

            stats_finish(st1_dram, r1_dram, q1_dram)

            # ================= steady phase =================
            G = 4   # states per tile group
            GD = 2  # states per broadcast DMA (descriptor-size tradeoff)
            NG = NS // G
            prev_h = None
            prev_cv = None
            for j in range(NCH):
                sl = slice(j * T, (j + 1) * T)
                bcr = j * 2 * NS  # chunk-major row base in bc_dram
                xj = wka.tile([C, T], F32, tag="xj")
                nc.sync.dma_start(xj[:], x_d[:, sl])
                rft = wka.tile([1, T], F32, tag="rft")
                nc.sync.dma_start(r32(rft[:]), r32(r1_dram[0:1, sl]))
                qft = wka.tile([1, T], F32, tag="qft")
                nc.sync.dma_start(r32(qft[:]), r32(q1_dram[0:1, sl]))

                rb = ps.tile([C, T], F32, tag="ps")
                nc.tensor.matmul(rb[:], ones1[:], r32(rft[:]), start=True, stop=True)
                xr = wka.tile([C, T], F32, tag="xr")
                nc.vector.tensor_mul(r32(xr[:]), xj[:], rb[:])

                xzh = ps.tile([D_INNER, T], F32, tag="ps")
                nc.tensor.matmul(xzh[:], r32(W2T[:, 0:D_INNER]), r32(xr[:]),
                                 start=True, stop=False, skip_group_check=True)
                nc.tensor.matmul(xzh[:], r32(w1n[:, 0:D_INNER]), r32(qft[:]),
                                 start=False, stop=True, skip_group_check=True)
                xzz = ps.tile([D_INNER, T], F32, tag="ps")
                nc.tensor.matmul(xzz[:], r32(W2T[:, D_INNER:2 * D_INNER]), r32(xr[:]),
                                 start=True, stop=False, skip_group_check=True)
                nc.tensor.matmul(xzz[:], r32(w1n[:, D_INNER:2 * D_INNER]), r32(qft[:]),
                                 start=False, stop=True, skip_group_check=True)

                cv = cvp.tile([D_INNER, T + 3], F32, tag="cv")
                if j == 0:
                    nc.vector.memset(cv[:, 0:3], 0.0)
                else:
                    nc.vector.tensor_copy(r32(cv[:, 0:3]), r32(prev_cv[:, T:T + 3]))
                nc.scalar.activation(r32(cv[:, 3:T + 3]), xzh[:], AF.Copy)
                cps = ps.tile([D_INNER, T], F32, tag="ps")
                for k in range(4):
                    nc.tensor.matmul(cps[:], r32(diag[:, k * D_INNER:(k + 1) * D_INNER]),
                                     r32(cv[:, k:k + T]),
                                     start=(k == 0), stop=(k == 3), skip_group_check=True)
                prev_cv = cv

                # both silus via sigmoid = exp(-ln(1+e^-u)), packed in one
                # [128,2,T] tile: single Ln site + single Exp site per chunk
                ezc = wka.tile([D_INNER, 2, T], F32, tag="tmpa")
                nc.scalar.activation(ezc[:, 0, :], xzz[:], AF.Exp, bias=bzn[:], scale=-1.0)
                nc.scalar.activation(ezc[:, 1, :], cps[:], AF.Exp, bias=cbn[:], scale=-1.0)
                nc.vector.tensor_scalar_add(ezc[:], ezc[:], 1.0)
                lzc = wka.tile([D_INNER, 2, T], F32, tag="tmpb")
                nc.scalar.activation(lzc[:], ezc[:], AF.Ln)
                nc.scalar.activation(lzc[:], lzc[:], AF.Exp, bias=0.0, scale=-1.0)
                xh = wka.tile([D_INNER, T], F32, tag="xh")
                nc.vector.scalar_tensor_tensor(r32(xh[:]), cps[:], cbp[:], lzc[:, 1, :],
                                               OP.add, OP.mult)
                sz = wka.tile([D_INNER, T], F32, tag="sz")
                nc.vector.scalar_tensor_tensor(sz[:], xzz[:], bzp[:], lzc[:, 0, :],
                                               OP.add, OP.mult)

                dblA = ps.tile([DT_RANK, T], F32, tag="ps")
                nc.tensor.matmul(dblA[:], r32(Wxp[:, 0:DT_RANK]), r32(xh[:]),
                                 start=True, stop=True)
                dblB = ps.tile([2 * NS, T], F32, tag="ps")
                nc.tensor.matmul(dblB[:], r32(Wxp[:, DT_RANK:]), r32(xh[:]),
                                 start=True, stop=True)
                dtin = wkb.tile([DT_RANK, T], F32, tag="dtin")
                nc.scalar.activation(r32(dtin[:]), dblA[:], AF.Copy)
                bc = wkb.tile([2 * NS, T], BF16, tag="bc")
                nc.scalar.copy(bc[:], dblB[:])
                nc.sync.dma_start(bc_dram[bcr:bcr + 2 * NS, :], bc[:])

                # broadcast B/C state-rows to all partitions; contiguous 2-row
                # groups in chunk-major bc_dram -> 1 descriptor per partition
                Bgs, Cgs = [], []
                for g in range(NG):
                    Bg = bgp.tile([D_INNER, G, T], BF16, tag="Bg")
                    for h in range(G // GD):
                        r0 = bcr + g * G + h * GD
                        nc.sync.dma_start(
                            Bg[:, h * GD:(h + 1) * GD, :],
                            bc_dram[r0:r0 + GD, :][None]
                            .broadcast_to([D_INNER, GD, T]))
                    Bgs.append(Bg)
                    Cg = bgp.tile([D_INNER, G, T], BF16, tag="Cg")
                    for h in range(G // GD):
                        r0 = bcr + NS + g * G + h * GD
                        nc.sync.dma_start(
                            Cg[:, h * GD:(h + 1) * GD, :],
                            bc_dram[r0:r0 + GD, :][None]
                            .broadcast_to([D_INNER, GD, T]))
                    Cgs.append(Cg)

                dtp = ps.tile([D_INNER, T], F32, tag="ps")
                nc.tensor.matmul(dtp[:], r32(WdtT[:]), r32(dtin[:]), start=True, stop=True)
                esp = wka.tile([D_INNER, T], F32, tag="tmpa")
                nc.scalar.activation(esp[:], dtp[:], AF.Exp, bias=dtb[:], scale=1.0)
                nc.vector.tensor_scalar_add(esp[:], esp[:], 1.0)
                dt = wka.tile([D_INNER, T], F32, tag="dt")
                nc.scalar.activation(dt[:], esp[:], AF.Ln)

                w = wkb.tile([D_INNER, T], BF16, tag="w")
                nc.vector.tensor_mul(w[:], dt[:], xh[:])

                cur_h = hp.tile([D_INNER, NS, T], BF16, tag="h")
                yacc = pacc.tile([D_INNER, T], F32, tag="yacc")
                for g in range(NG):
                    dAg = dap.tile([D_INNER, G, T], F32, tag="dAg")
                    for k in range(G):
                        n = g * G + k
                        nc.scalar.activation(dAg[:, k, :], dt[:], AF.Exp, bias=0.0,
                                             scale=Aneg[:, n:n + 1])
                    dBg = dbp.tile([D_INNER, G, T], BF16, tag="dBg")
                    nc.vector.tensor_tensor(dBg[:], w[:][:, None].broadcast_to(
                        [D_INNER, G, T]), Bgs[g][:], OP.mult)
                    for k in range(G):
                        n = g * G + k
                        init = 0.0 if j == 0 else prev_h[:, n, T - 1:T]
                        nc.vector.tensor_tensor_scan(cur_h[:, n, :], dAg[:, k, :],
                                                     dBg[:, k, :], init,
                                                     OP.mult, OP.add)
                    Pg = dbp.tile([D_INNER, G, T], BF16, tag="Pg")
                    nc.vector.tensor_tensor(Pg[:], cur_h[:, g * G:(g + 1) * G, :],
                                            Cgs[g][:], OP.mult)
                    for k in range(G):
                        n = g * G + k
                        nc.tensor.matmul(yacc[:], I128[:], Pg[:, k, :], start=(n == 0),
                                         stop=(n == NS - 1), skip_group_check=True)
                prev_h = cur_h

                y = wka.tile([D_INNER, T], F32, tag="y")
                nc.vector.scalar_tensor_tensor(y[:], xh[:], Dsk[:], yacc[:],
                                               OP.mult, OP.add)
                gated = wka.tile([D_INNER, T], F32, tag="gated")
                nc.vector.tensor_mul(r32(gated[:]), y[:], sz[:])
                opj = pso.tile([C, T], F32, tag="pso")
                nc.tensor.matmul(opj[:], r32(WoutT[:]), r32(gated[:]), start=True, stop=True)
                nc.vector.tensor_add(fbuf[:, sl], xj[:], opj[:])

                f2 = wka.tile([C, T], F32, tag="f2")
                nc.scalar.activation(r32(f2[:]), fbuf[:, sl], AF.Square)
                mu2_ps = pso.tile([1, T], F32, tag="pso")
                nc.tensor.matmul(mu2_ps[:], mvb[:], fbuf[:, sl], start=True, stop=True)
                ms2_ps = pso.tile([1, T], F32, tag="pso")
                nc.tensor.matmul(ms2_ps[:], mv[:], r32(f2[:]), start=True, stop=True)
                mu2_sb = wka.tile([1, T], F32, tag="msb")
                nc.scalar.copy(mu2_sb[:], mu2_ps[:])
                ms2_sb = wka.tile([1, T], F32, tag="ssb")
                nc.scalar.copy(ms2_sb[:], ms2_ps[:])
                nc.sync.dma_start(st2_dram[j:j + 1, :], mu2_sb[:])
                nc.sync.dma_start(st2_dram[16 + j:17 + j, :], ms2_sb[:])

            stats_finish(st2_dram, r2_dram, q2_dram)

            # ================= final phase: LN2 + MLP (gelu table) =========
            for j in range(NCH):
                sl = slice(j * T, (j + 1) * T)
                rft2 = wka.tile([1, T], F32, tag="rft")
                nc.sync.dma_start(r32(rft2[:]), r32(r2_dram[0:1, sl]))
                qft2 = wka.tile([1, T], F32, tag="qft")
                nc.sync.dma_start(r32(qft2[:]), r32(q2_dram[0:1, sl]))
                rb2 = ps.tile([C, T], F32, tag="ps")
                nc.tensor.matmul(rb2[:], ones1[:], r32(rft2[:]), start=True, stop=True)
                fr = wka.tile([C, T], F32, tag="fr")
                nc.vector.tensor_mul(r32(fr[:]), fbuf[:, sl], rb2[:])
                gtiles = []
                for h in range(2):
                    gp = ps.tile([2 * C, T], F32, tag="ps")
                    nc.tensor.matmul(gp[:], r32(Wfc1T[:, h * 2 * C:(h + 1) * 2 * C]),
                                     r32(fr[:]), start=True, stop=False,
                                     skip_group_check=True)
                    nc.tensor.matmul(gp[:], r32(w1f[:, h * 2 * C:(h + 1) * 2 * C]),
                                     r32(qft2[:]), start=False, stop=True,
                                     skip_group_check=True)
                    g = wka.tile([2 * C, T], F32, tag="g")
                    nc.scalar.activation(r32(g[:]), gp[:], AF.Gelu, bias=bfc1[:, h:h + 1])
                    gtiles.append(g)
                f2p = ps.tile([C, T], F32, tag="ps")
                nc.tensor.matmul(f2p[:], r32(Wfc2a[:]), r32(gtiles[0][:]),
                                 start=True, stop=False, skip_group_check=True)
                nc.tensor.matmul(f2p[:], r32(Wfc2b[:]), r32(gtiles[1][:]),
                                 start=False, stop=True, skip_group_check=True)
                outf = wka.tile([C, T], F32, tag="outf")
                nc.vector.tensor_add(outf[:], fbuf[:, sl], f2p[:])
                nc.sync.dma_start(out_d[:, sl], outf[:])

    nc.compile()
    return nc


def _get_program():
    if "nc" not in _PROG:
        _PROG["nc"] = _build_program()
    return _PROG["nc"]


# ----------------------------------------------------------------------------
# host weight preprocessing per direction
# ----------------------------------------------------------------------------

def _bf16_dtype():
    try:
        import ml_dtypes
        return ml_dtypes.bfloat16
    except ImportError:
        import jax.numpy as jnp
        return jnp.bfloat16


def _prep_weights(li, inputs):
    f32 = np.float32
    in_w = np.asarray(inputs["in_proj_w"][li], np.float64)
    nw = np.asarray(inputs["norm_w"][li], np.float64)
    nb = np.asarray(inputs["norm_b"][li], np.float64)
    W2 = in_w * nw[None, :]
    bz_full = in_w @ nb
    b_h, b_z = bz_full[:D_INNER], bz_full[D_INNER:]
    cw = np.asarray(inputs["conv_w"][li], np.float64)
    cb = np.asarray(inputs["conv_b"][li], np.float64)
    cbtot = cb + b_h * cw.sum(1)
    diag = np.zeros((D_INNER, 4, D_INNER), np.float64)
    kk = np.arange(D_INNER)
    for k in range(4):
        diag[kk, k, kk] = cw[:, k]
    xp = np.asarray(inputs["x_proj_w"][li], np.float64)
    fc1 = np.asarray(inputs["fc1_w"], np.float64)
    fw = np.asarray(inputs["fnorm_w"], np.float64)
    fb = np.asarray(inputs["fnorm_b"], np.float64)
    fc1p = fc1 * fw[None, :]
    bfc1 = fc1 @ fb
    fc2 = np.asarray(inputs["fc2_w"], np.float64)
    return {
        "W2T": W2.T.astype(f32),
        "w1n": (-W2.sum(1))[None, :].astype(f32),
        "diag": diag.reshape(D_INNER, 4 * D_INNER).astype(f32),
        "cbn": (-cbtot)[:, None].astype(f32),
        "cbp": cbtot[:, None].astype(f32),
        "bzn": (-b_z)[:, None].astype(f32),
        "bzp": b_z[:, None].astype(f32),
        "Wxp": xp.T.astype(f32),
        "WdtT": np.asarray(inputs["dt_proj_w"][li], np.float64).T.astype(f32),
        "dtb": np.asarray(inputs["dt_proj_b"][li], f32)[:, None],
        "Aneg": (-np.exp(np.asarray(inputs["A_log"][li], np.float64))).astype(f32),
        "Dsk": np.asarray(inputs["D_skip"][li], f32)[:, None],
        "WoutT": np.asarray(inputs["out_proj_w"][li], np.float64).T.astype(f32),
        "Wfc1T": fc1p.T.astype(f32),
        "w1f": (-fc1p.sum(1))[None, :].astype(f32),
        "bfc1": bfc1.reshape(2, 128).T.astype(f32),
        "Wfc2a": fc2.T[:2 * C].astype(f32),
        "Wfc2b": fc2.T[2 * C:].astype(f32),
        "mv": np.full((C, 1), 1.0 / C, f32),
        "mvb": np.full((C, 1), 1.0 / C).astype(_bf16_dtype()),
        "ones1": np.ones((1, C), f32),
        "epsb": np.full((16, 1), EPS, f32),
        "I128": np.eye(D_INNER).astype(_bf16_dtype()),
    }


def _reference_np(**inputs):
    """Pure-numpy fallback replica of the reference (slow, exact)."""
    i = int(inputs["src_number"]) % 4
    s = int(inputs["step_size"])
    feats, ori_h, ori_w = _rcds_np(np.asarray(inputs["ref_feat"], np.float32),
                                   np.asarray(inputs["src_feat"], np.float32), s, i)
    Bb, K, Cc, Ll = feats.shape
    f = feats.astype(np.float64)
    outs = np.empty_like(f)
    for d in range(4):
        li = d
        x = f[:, d].transpose(0, 2, 1)  # (B,L,C)
        mu = x.mean(-1, keepdims=True)
        var = ((x - mu) ** 2).mean(-1, keepdims=True)
        h = (x - mu) / np.sqrt(var + EPS) * np.asarray(inputs["norm_w"][li]) \
            + np.asarray(inputs["norm_b"][li])
        xz = h @ np.asarray(inputs["in_proj_w"][li]).T
        xh, z = xz[..., :D_INNER], xz[..., D_INNER:]
        xpd = np.pad(xh.transpose(0, 2, 1), ((0, 0), (0, 0), (3, 0)))
        cw = np.asarray(inputs["conv_w"][li])
        xc = sum(cw[:, k:k + 1] * xpd[:, :, k:k + Ll] for k in range(4))
        xc = xc + np.asarray(inputs["conv_b"][li])[None, :, None]
        xh = (xc / (1 + np.exp(-xc))).transpose(0, 2, 1)
        dbl = xh @ np.asarray(inputs["x_proj_w"][li]).T
        dtv = dbl[..., :DT_RANK]
        Bm = dbl[..., DT_RANK:DT_RANK + D_STATE]
        Cm = dbl[..., DT_RANK + D_STATE:]
        dtp = dtv @ np.asarray(inputs["dt_proj_w"][li]).T + np.asarray(inputs["dt_proj_b"][li])
        dtv = np.logaddexp(0, dtp)
        A = -np.exp(np.asarray(inputs["A_log"][li], np.float64))
        dA = np.exp(dtv[..., None] * A)
        dBu = (dtv * xh)[..., None] * Bm[:, :, None, :]
        hst = np.zeros((Bb, D_INNER, D_STATE))
        ys = np.empty((Bb, Ll, D_INNER))
        for t in range(Ll):
            hst = dA[:, t] * hst + dBu[:, t]
            ys[:, t] = np.einsum("bdn,bn->bd", hst, Cm[:, t])
        ys = ys + xh * np.asarray(inputs["D_skip"][li])
        ys = ys * (z / (1 + np.exp(-z)))
        o = ys @ np.asarray(inputs["out_proj_w"][li]).T
        outs[:, d] = (x + o).transpose(0, 2, 1)
    x = outs.transpose(0, 1, 3, 2)  # (B,4,L,C)
    mu = x.mean(-1, keepdims=True)
    var = ((x - mu) ** 2).mean(-1, keepdims=True)
    h = (x - mu) / np.sqrt(var + EPS) * np.asarray(inputs["fnorm_w"]) \
        + np.asarray(inputs["fnorm_b"])
    from scipy.special import erf
    g = h @ np.asarray(inputs["fc1_w"]).T
    g = 0.5 * g * (1 + erf(g / np.sqrt(2)))
    x = x + g @ np.asarray(inputs["fc2_w"]).T
    d0, d1 = _merge_np(x.transpose(0, 1, 3, 2).astype(np.float32), ori_h, ori_w, s, i)
    return d0.astype(np.float32), d1.astype(np.float32)


def kernel(**inputs):
    s = int(inputs["step_size"])
    i = int(inputs["src_number"]) % 4
    ref_feat = np.asarray(inputs["ref_feat"], np.float32)
    src_feat = np.asarray(inputs["src_feat"], np.float32)

    if s != 2 or ref_feat.shape != (B, C, HIMG, WIMG):
        return _reference_np(**inputs)

    feats, ori_h, ori_w = _rcds_np(ref_feat, src_feat, s, i)  # (B,4,C,L)

    wmaps = [_prep_weights(d, inputs) for d in range(4)]
    in_maps = []
    for core in range(8):
        b, d = core // 4, core % 4
        m = dict(wmaps[d])
        m["x"] = np.ascontiguousarray(feats[b, d])
        in_maps.append(m)

    from concourse.bass_utils import run_bass_kernel_spmd
    nc = _get_program()
    res = run_bass_kernel_spmd(nc, in_maps, list(range(8)))

    ys = np.empty((B, 4, C, L), np.float32)
    for core in range(8):
        b, d = core // 4, core % 4
        ys[b, d] = res.results[core]["out"]

    d0, d1 = _merge_np(ys, ori_h, ori_w, s, i)
    return d0, d1

